# revision 24
# baseline (speedup 1.0000x reference)
"""CTEG kernel for 8x TRN2 NeuronCores.

K1 (SPMD, 8 cores): data-parallel recurrence (2 batch rows/core): encoder
   (bi-LSTM over T=8) + 64-step decoder with memory network + attention,
   emitting decoder hidden states hs [64, 2, 512].
K2 (SPMD, 8 cores): vocab-sharded projection: each core computes
   logits[:, :, c*4000:(c+1)*4000] = hs_all @ Wout_c.T + bout_c.

Host side: embedding gathers, weight transposes, shard assembly.
"""

import sys

sys.path.insert(0, "/opt/trn_rl_repo")

from contextlib import ExitStack

import numpy as np

import concourse.bass as bass
import concourse.mybir as mybir
import concourse.tile as tile
from concourse.masks import make_identity

B, T, L, V, E, H, A, M = 16, 8, 64, 32000, 300, 512, 128, 120
NC = 8
BL = B // NC          # 2 batch rows per core
VS = V // NC          # 4000 vocab rows per core
F32 = mybir.dt.float32
F32R = mybir.dt.float32  # fp32r needs rounded producers; plain fp32 for now
AF = mybir.ActivationFunctionType
MEMC = 256            # B*M=240 padded to 256 (fp32r needs free>=256 for 1cyc/row)
ECH = [(0, 128), (128, 256), (256, 300)]             # E row chunks
EACH = [(0, 128), (128, 256), (256, 301)]            # E+1 (bias row) chunks
HCH = [(0, 128), (128, 256), (256, 384), (384, 512)]

_cache = {}


def _chunked_load(nc, pool, dram, chunks, ncols, tag, dtype=F32R):
    # dram is padded to len(chunks)*128 rows; single DMA, chunk-major layout
    nch = len(chunks)
    t_ = pool.tile([128, nch, ncols], dtype, tag=tag)
    src = dram[0 : 128 * nch, :].rearrange("(c p) n -> p c n", p=128)
    if dtype == F32R:
        src = src.bitcast(F32R)
    nc.sync.dma_start(t_, src)
    return t_


def build_k1(steps=L, tsteps=T, fused=False):
    nc = bass.Bass(trn_type="TRN2", name="cteg_fused" if fused else "cteg_rec",
                   num_devices=NC if fused else None)
    d = {}

    def inp(name, shape):
        d[name] = nc.dram_tensor(name, list(shape), F32, kind="ExternalInput")
        return d[name]

    TB = 2 * tsteps
    inp("topicT_a", (384, TB))
    inp("essayT_a", (384, 2 * steps))
    inp("memT0", (384, MEMC))
    inp("enc_xT_f", (384, 4 * H))
    inp("enc_xT_b", (384, 4 * H))
    inp("enc_hT_f", (H, 4 * H))
    inp("enc_hT_b", (H, 4 * H))
    inp("decXT", (384, 4 * H))
    inp("decHT", (H, 4 * H))
    inp("decMT", (384, 4 * H))
    inp("decAT", (H, 4 * H))
    inp("wp1T_a", (640, E))
    inp("wp2T_a", (640, A))
    inp("wepT_a", (640, A))
    inp("wi1T_a", (384, E))
    inp("wmpT", (384, E))
    inp("attn_vT", (A, 1))
    inp("mask_attn", (TB, BL))      # [(t,b), b'] = (b==b')
    inp("mask_memT", (BL, MEMC))    # [b', c] = (c//120==b'), pad cols 0
    if fused:
        d["woT"] = nc.dram_tensor("woT", [640, VS], mybir.dt.float16,
                                  kind="ExternalInput")
        # int8 logits + per-(row, col-group) absmax scales; host dequantizes
        lg = nc.dram_tensor("lg", [B * L, VS], mybir.dt.int8,
                            kind="ExternalOutput")
        lsc = nc.dram_tensor("lsc", [B * L, 2], F32, kind="ExternalOutput")
    else:
        hs = nc.dram_tensor("hs", [steps, BL, H], F32, kind="ExternalOutput")

    with tile.TileContext(nc) as tc:
        with ExitStack() as ctx:
            wp = ctx.enter_context(tc.tile_pool(name="wts", bufs=1))
            sp = ctx.enter_context(tc.tile_pool(name="big", bufs=1))
            stp = ctx.enter_context(tc.tile_pool(name="state", bufs=3))
            rp = ctx.enter_context(tc.tile_pool(name="roll", bufs=4))
            sgp = ctx.enter_context(tc.tile_pool(name="sigp", bufs=2))
            # recurrence-phase pools (PSUM + decoder weights); closed before
            # the fused vocab-projection phase to free PSUM banks and SBUF
            rctx = ExitStack()
            pg = rctx.enter_context(tc.tile_pool(name="psg", bufs=1, space="PSUM"))
            pb = rctx.enter_context(tc.tile_pool(name="psb", bufs=1, space="PSUM"))
            pt = rctx.enter_context(tc.tile_pool(name="pst", bufs=1, space="PSUM"))

            # ---- small resident constants ----
            topicT = _chunked_load(nc, wp, d["topicT_a"], EACH, TB, "topicT")
            essayT = _chunked_load(nc, wp, d["essayT_a"], EACH, 2 * steps, "essayT")
            HACH = [(0, 128), (128, 256), (256, 384), (384, 512), (512, 513)]
            wp1T = _chunked_load(nc, wp, d["wp1T_a"], HACH, E, "wp1T")
            wp2T = _chunked_load(nc, wp, d["wp2T_a"], HACH, A, "wp2T")
            wepT = _chunked_load(nc, wp, d["wepT_a"], HACH, A, "wepT")
            wi1T = _chunked_load(nc, wp, d["wi1T_a"], EACH, E, "wi1T")
            wmpT = _chunked_load(nc, wp, d["wmpT"], ECH, E, "wmpT")
            attn_vT = wp.tile([A, 1], F32R, tag="attn_vT")
            nc.sync.dma_start(attn_vT, d["attn_vT"][:, :].bitcast(F32R))
            mask_attn = wp.tile([TB, BL], F32, tag="mask_attn")
            nc.sync.dma_start(mask_attn, d["mask_attn"][:, :])
            mask_memT = wp.tile([BL, MEMC], F32, tag="mask_memT")
            nc.sync.dma_start(mask_memT, d["mask_memT"][:, :])
            mask_memTr = mask_memT.bitcast(F32R)

            ident = wp.tile([128, 128], F32, tag="ident")
            make_identity(nc, ident)
            identr = ident.bitcast(F32R)
            ones2f = wp.tile([2, 128], F32, tag="ones2")
            nc.vector.memset(ones2f, 1.0)
            ones2 = ones2f.bitcast(F32R)

            memT = sp.tile([128, 3, MEMC], F32, tag="memT")
            nc.sync.dma_start(
                memT, d["memT0"][0:384, :].rearrange("(c p) n -> p c n", p=128))
            memTr = memT.bitcast(F32R)

            h_bm = stp.tile([2, H], F32, tag="h_bm")
            c_bm = stp.tile([2, H], F32, tag="c_bm")
            # enc_outs stored transposed: eoT[:, k, 2t+b] = enc_outs[b, t, 128k+p]
            eoT = sp.tile([128, 4, TB], F32, tag="eoT")
            if fused:
                # hsT_acc[p, k, b*steps+t] = dec h_t[b, 128k+p] (f16 for the
                # fp16 vocab projection; recurrence itself stays fp32)
                hsT_acc = sp.tile([128, 4, BL * steps], mybir.dt.float16,
                                  tag="hsT_acc")

            def lstm_pointwise(gate_ps, cprev, cnext, hnext):
                # gate_ps [2, 4H] flat: i|f|g|o
                sig = sgp.tile([2, 4 * H], F32, tag="sig")
                nc.scalar.activation(sig[:, 0 : 2 * H], gate_ps[:, 0 : 2 * H],
                                     AF.Sigmoid)
                nc.scalar.activation(sig[:, 2 * H : 3 * H],
                                     gate_ps[:, 2 * H : 3 * H], AF.Tanh)
                nc.scalar.activation(sig[:, 3 * H : 4 * H],
                                     gate_ps[:, 3 * H : 4 * H], AF.Sigmoid)
                tmp = rp.tile([2, H], F32, tag="ctmp")
                nc.vector.tensor_mul(cnext, sig[:, H : 2 * H], cprev)
                nc.vector.tensor_mul(tmp, sig[:, 0:H], sig[:, 2 * H : 3 * H])
                nc.vector.tensor_add(cnext, cnext, tmp)
                tc2 = rp.tile([2, H], F32, tag="tc2")
                nc.scalar.activation(tc2, cnext, AF.Tanh)
                nc.vector.tensor_mul(hnext, sig[:, 3 * H : 4 * H], tc2)

            # ================= ENCODER =================
            hfin = {}
            cfin = {}
            with ExitStack() as ectx:
                eps2 = ectx.enter_context(tc.tile_pool(name="encs", bufs=4))
                for dr in ("f", "b"):
                    with ExitStack() as dctx:
                        epd = dctx.enter_context(
                            tc.tile_pool(name=f"encw{dr}", bufs=1))
                        xsb = epd.tile([TB, 4 * H], F32R, tag="xsb")
                        with ExitStack() as xctx:
                            xp = xctx.enter_context(
                                tc.tile_pool(name=f"encx{dr}", bufs=1))
                            xpp = xctx.enter_context(
                                tc.tile_pool(name=f"encxp{dr}", bufs=1,
                                             space="PSUM"))
                            ew = _chunked_load(nc, xp, d[f"enc_xT_{dr}"], EACH,
                                               4 * H, "ew")
                            for hf_ in range(2):
                                xps = xpp.tile([TB, 2 * H], F32, tag="xps")
                                for ki, (r0, r1) in enumerate(EACH):
                                    for c2 in range(2):
                                        cc = 2 * hf_ + c2
                                        nc.tensor.matmul(
                                            xps[:, 512 * c2 : 512 * c2 + 512],
                                            topicT[: r1 - r0, ki, :],
                                            ew[: r1 - r0, ki,
                                               512 * cc : 512 * cc + 512],
                                            start=(ki == 0), stop=(ki == 2))
                                nc.scalar.copy(
                                    xsb.bitcast(F32)[:, 1024 * hf_ :
                                                     1024 * hf_ + 1024], xps)
                        ehw = _chunked_load(
                            nc, epd, d[f"enc_hT_{dr}"],
                            [(128 * k, 128 * k + 128) for k in range(4)],
                            4 * H, "ehw")
                        hT0 = eps2.tile([128, 4, 2], F32R, tag="ehT")
                        nc.vector.memset(hT0.bitcast(F32), 0.0)
                        hT = None
                        cd = eps2.tile([2, H], F32, tag="ecd")
                        nc.vector.memset(cd, 0.0)
                        for s in range(tsteps):
                            t = s if dr == "f" else tsteps - 1 - s
                            tc.strict_bb_all_engine_barrier()
                            gps = pg.tile([2, 4 * H], F32, tag="gps")
                            if s == 0:
                                hT_prev = hT0
                            elif dr == "f":
                                hT_prev = eoT.bitcast(F32R)[
                                    :, :, 2 * (t - 1) : 2 * (t - 1) + 2]
                            else:
                                hT_prev = hT
                            for cc in range(4):
                                cs = slice(512 * cc, 512 * cc + 512)
                                for ki in range(4):
                                    nc.tensor.matmul(
                                        gps[:, cs],
                                        hT_prev[:, ki, :], ehw[:, ki, cs],
                                        start=(ki == 0), stop=False)
                                nc.tensor.matmul(
                                    gps[:, cs],
                                    identr[:TB, 2 * t : 2 * t + 2],
                                    xsb[:, cs],
                                    start=False, stop=True)
                            cnew = eps2.tile([2, H], F32, tag="ecn")
                            hnew = eps2.tile([2, H], F32, tag="ehn")
                            lstm_pointwise(gps, cd, cnew, hnew)
                            cd = cnew
                            tp = pt.tile([128, 8], F32, tag="tp")
                            for k, (r0, r1) in enumerate(HCH):
                                nc.tensor.transpose(
                                    tp[:, 2 * k : 2 * k + 2],
                                    hnew[:, r0:r1], ident[:2, :2])
                            tdst = eoT[:, :, 2 * t : 2 * t + 2]
                            tsrc = tp.rearrange("p (k b) -> p k b", b=2)
                            if dr == "f":
                                nc.vector.tensor_copy(tdst, tsrc)
                            else:
                                nc.vector.tensor_add(tdst, tdst, tsrc)
                            if s < tsteps - 1:
                                if dr == "f":
                                    hT = None  # fwd reads eoT directly
                                else:
                                    hT = eps2.tile([128, 4, 2], F32R, tag="ehT")
                                    nc.vector.tensor_copy(hT.bitcast(F32), tsrc)
                            else:
                                hfin[dr] = hnew
                        cfin[dr] = cd
                nc.vector.tensor_add(h_bm, hfin["f"], hfin["b"])
                nc.vector.tensor_add(c_bm, cfin["f"], cfin["b"])

            # dec weights in a pool opened after encoder pools closed
            H4CH = [(128 * k, 128 * k + 128) for k in range(4)]
            dwp = rctx.enter_context(tc.tile_pool(name="decw", bufs=1))
            decXT = _chunked_load(nc, dwp, d["decXT"], EACH, 4 * H, "decXT")
            decHT = _chunked_load(nc, dwp, d["decHT"], H4CH, 4 * H, "decHT")
            decMT = _chunked_load(nc, dwp, d["decMT"], ECH, 4 * H, "decMT")

            # hcT: chunks 0-3 = hT, 4-7 = cT
            hcT = stp.tile([128, 8, 2], F32R, tag="hcT")
            tp0 = pt.tile([128, 16], F32, tag="tp")
            for k, (r0, r1) in enumerate(HCH):
                nc.tensor.transpose(tp0[:, 2 * k : 2 * k + 2], h_bm[:, r0:r1],
                                    ident[:2, :2])
                nc.tensor.transpose(tp0[:, 8 + 2 * k : 8 + 2 * k + 2],
                                    c_bm[:, r0:r1], ident[:2, :2])
            nc.vector.tensor_copy(hcT.bitcast(F32),
                                  tp0.rearrange("p (k b) -> p k b", b=2))

            tc.strict_bb_all_engine_barrier()
            # ---- precompute phase ----
            TBL = 2 * steps
            P_sb = sp.tile([TB, 4 * H], F32R, tag="P_sb")
            epT_sb = sp.tile([A, TB], F32, tag="epT_sb")
            UT_sb = sp.tile([128, 3, TBL], F32, tag="UT_sb")
            XD_sb = sp.tile([TBL, 4 * H], F32R, tag="XD_sb")
            with ExitStack() as pctx:
                ppre = pctx.enter_context(
                    tc.tile_pool(name="pre", bufs=1, space="PSUM"))
                dap = pctx.enter_context(tc.tile_pool(name="decA", bufs=1))
                for hf_ in range(2):
                    decAT = dap.tile([128, 4, 1024], F32R, tag="decAT")
                    nc.sync.dma_start(
                        decAT,
                        d["decAT"][:, 1024 * hf_ : 1024 * hf_ + 1024].rearrange(
                            "(c p) n -> p c n", p=128).bitcast(F32R))
                    pps = ppre.tile([TB, 2 * H], F32, tag="pre")
                    for ki in range(4):
                        for c2 in range(2):
                            nc.tensor.matmul(
                                pps[:, 512 * c2 : 512 * c2 + 512], eoT.bitcast(F32R)[:, ki, :],
                                decAT[:, ki, 512 * c2 : 512 * c2 + 512],
                                start=(ki == 0), stop=(ki == 3))
                    nc.scalar.copy(
                        P_sb.bitcast(F32)[:, 1024 * hf_ : 1024 * hf_ + 1024], pps)

                # enc_procT [A, TB] (A-major): lhsT = wepT chunks, rhs = eoT (+ones)
                eph = ppre.tile([A, TB], F32, tag="pre")
                for ki in range(4):
                    nc.tensor.matmul(eph, wepT[:, ki, :], eoT.bitcast(F32R)[:, ki, :],
                                     start=(ki == 0), stop=False)
                nc.tensor.matmul(eph, wepT[0:1, 4, :], ones2[0:1, 0:TB],
                                 start=False, stop=True)
                nc.vector.tensor_copy(epT_sb, eph)

                for j, (c0, c1) in enumerate(ECH):
                    ups = ppre.tile([128, TBL], F32, tag="pre")
                    for ki, (r0, r1) in enumerate(EACH):
                        nc.tensor.matmul(ups[: c1 - c0, :],
                                         wi1T[: r1 - r0, ki, c0:c1],
                                         essayT[: r1 - r0, ki, :],
                                         start=(ki == 0), stop=(ki == 2))
                    nc.scalar.copy(UT_sb[: c1 - c0, j, :], ups[: c1 - c0, :])

                for cc in range(4):
                    xps2 = ppre.tile([TBL, H], F32, tag="pre")
                    for ki, (r0, r1) in enumerate(EACH):
                        nc.tensor.matmul(xps2, essayT[: r1 - r0, ki, :],
                                         decXT[: r1 - r0, ki,
                                               512 * cc : 512 * cc + 512],
                                         start=(ki == 0), stop=(ki == 2))
                    nc.scalar.copy(XD_sb.bitcast(F32)[:, 512 * cc : 512 * cc + 512],
                                   xps2)

            pc = rctx.enter_context(tc.tile_pool(name="psc", bufs=1, space="PSUM"))

            # ================= DECODER =================
            for t in range(steps):
                tc.strict_bb_all_engine_barrier()
                # ---- mem write pipeline (h-independent) ----
                candp = pc.tile([128, 3, MEMC], F32, tag="candp")
                for j, (c0, c1) in enumerate(ECH):
                    for ki, (r0, r1) in enumerate(ECH):
                        nc.tensor.matmul(candp[: c1 - c0, j, :],
                                         wmpT[: r1 - r0, ki, c0:c1],
                                         memTr[: r1 - r0, ki, :],
                                         start=(ki == 0), stop=(ki == 2))
                gps_m = pb.tile([2, MEMC], F32, tag="sm")
                for ki, (r0, r1) in enumerate(ECH):
                    nc.tensor.matmul(gps_m, essayT[: r1 - r0, ki, 2 * t : 2 * t + 2],
                                     memTr[: r1 - r0, ki, :],
                                     start=(ki == 0), stop=(ki == 2))
                g_sb = rp.tile([2, MEMC], F32, tag="g_sb")
                nc.scalar.activation(g_sb, gps_m, AF.Sigmoid)
                nc.vector.tensor_mul(g_sb, g_sb, mask_memT)

                tc.strict_bb_all_engine_barrier()
                # ---- mem read: v, sim, mt ----
                vps = pb.tile([2, E], F32, tag="sm")
                for ki in range(4):
                    nc.tensor.matmul(vps, hcT[:, ki, :], wp1T[:, ki, :],
                                     start=(ki == 0), stop=False)
                nc.tensor.matmul(vps, ones2[0:1, 0:2], wp1T[0:1, 4, :],
                                 start=False, stop=True)
                v_bm = rp.tile([2, E], F32, tag="v_bm")
                nc.scalar.activation(v_bm, vps, AF.Tanh)
                vT = rp.tile([128, 3, 2], F32R, tag="vT")
                tpv = pt.tile([128, 6], F32, tag="tp")
                for j, (r0, r1) in enumerate(ECH):
                    nc.tensor.transpose(tpv[: r1 - r0, 2 * j : 2 * j + 2],
                                        v_bm[:, r0:r1], ident[:2, :2])
                for j, (r0, r1) in enumerate(ECH):
                    nc.vector.tensor_copy(vT.bitcast(F32)[: r1 - r0, j, :],
                                          tpv[: r1 - r0, 2 * j : 2 * j + 2])
                sps = pb.tile([2, MEMC], F32, tag="sm")
                for ki, (r0, r1) in enumerate(ECH):
                    nc.tensor.matmul(sps, vT[: r1 - r0, ki, :],
                                     memTr[: r1 - r0, ki, :],
                                     start=(ki == 0), stop=(ki == 2))
                es = rp.tile([2, MEMC], F32, tag="es")
                nc.scalar.activation(es, sps, AF.Exp)
                den = rp.tile([2, 1], F32, tag="den")
                nc.vector.tensor_mul(es, es, mask_memT)
                nc.vector.tensor_reduce(op=mybir.AluOpType.add, out=den,
                                        in_=es, axis=mybir.AxisListType.X)
                nc.vector.reciprocal(den, den)
                nc.vector.tensor_scalar_mul(es, es, den)
                esr = es.bitcast(F32R)

                tc.strict_bb_all_engine_barrier()
                mtT = rp.tile([128, 3, 2], F32R, tag="mtT")
                junk = rp.tile([128, 120], F32, tag="junk")
                for j, (r0, r1) in enumerate(ECH):
                    arep = pb.tile([128, MEMC], F32, tag="sm")
                    nc.tensor.matmul(arep[: r1 - r0, :], ones2[:, : r1 - r0], esr,
                                     start=True, stop=True)
                    for b in range(2):
                        nc.vector.tensor_mul(
                            junk[: r1 - r0, :],
                            memT[: r1 - r0, j, 120 * b : 120 * b + 120],
                            arep[: r1 - r0, 120 * b : 120 * b + 120])
                        nc.vector.tensor_reduce(
                            op=mybir.AluOpType.add,
                            out=mtT.bitcast(F32)[: r1 - r0, j, b : b + 1],
                            in_=junk[: r1 - r0, :], axis=mybir.AxisListType.X)

                tc.strict_bb_all_engine_barrier()
                # ---- attention ----
                qps = pb.tile([A, 2], F32, tag="sm")
                for ki in range(4):
                    nc.tensor.matmul(qps, wp2T[:, ki, :], hcT[:, 4 + ki, :],
                                     start=(ki == 0), stop=False)
                nc.tensor.matmul(qps, wp2T[0:1, 4, :], ones2[0:1, 0:2],
                                 start=False, stop=True)
                qsb = rp.tile([A, 2], F32, tag="qsb")
                nc.vector.tensor_copy(qsb, qps)
                tha = rp.tile([A, TB], F32, tag="tha")
                for b in range(2):
                    nc.scalar.activation(
                        tha.rearrange("a (t b) -> a t b", b=2)[:, :, b],
                        epT_sb.rearrange("a (t b) -> a t b", b=2)[:, :, b],
                        AF.Tanh, bias=qsb[:, b : b + 1], scale=1.0)
                scps = pb.tile([1, TB], F32, tag="sm")
                nc.tensor.matmul(scps, attn_vT, tha.bitcast(F32R),
                                 start=True, stop=True)
                esc = rp.tile([1, TB], F32, tag="esc")
                nc.scalar.activation(esc, scps, AF.Exp)
                escT = pt.tile([TB, 1], F32, tag="tp")
                nc.tensor.transpose(escT, esc, ident[0:1, 0:1])
                escTs = rp.tile([TB, 1], F32, tag="escTs")
                nc.vector.tensor_copy(escTs, escT)
                sms = pb.tile([2, 1], F32, tag="sm")
                nc.tensor.matmul(sms, mask_attn.bitcast(F32R),
                                 escTs.bitcast(F32R), start=True, stop=True)
                rden = rp.tile([2, 1], F32, tag="rden")
                nc.vector.reciprocal(rden, sms)
                rrT = pt.tile([1, 2], F32, tag="tp")
                nc.tensor.transpose(rrT, rden, ident[:2, :2])
                rr_sb = rp.tile([1, 2], F32, tag="rr_sb")
                nc.vector.tensor_copy(rr_sb, rrT)
                rrep = pb.tile([TB, 2], F32, tag="sm")
                nc.tensor.matmul(rrep, ones2[0:1, 0:TB], rr_sb.bitcast(F32R),
                                 start=True, stop=True)
                alBD = rp.tile([TB, BL], F32, tag="alBD")
                nc.vector.tensor_scalar_mul(alBD, mask_attn, escTs)
                nc.vector.tensor_mul(alBD, alBD, rrep)

                tc.strict_bb_all_engine_barrier()
                # ---- gates ----
                gps = pg.tile([2, 4 * H], F32, tag="gps")
                for cc in range(4):
                    cs = slice(512 * cc, 512 * cc + 512)
                    for ki in range(4):
                        nc.tensor.matmul(gps[:, cs],
                                         hcT[:, ki, :], decHT[:, ki, cs],
                                         start=(ki == 0), stop=False)
                    for ki, (r0, r1) in enumerate(ECH):
                        nc.tensor.matmul(gps[:, cs],
                                         mtT[: r1 - r0, ki, :],
                                         decMT[: r1 - r0, ki, cs],
                                         start=False, stop=False)
                    nc.tensor.matmul(gps[:, cs],
                                     alBD.bitcast(F32R), P_sb[:, cs],
                                     start=False, stop=False)
                    nc.tensor.matmul(gps[:, cs],
                                     identr[:TBL, 2 * t : 2 * t + 2],
                                     XD_sb[:, cs],
                                     start=False, stop=True)

                c_new = stp.tile([2, H], F32, tag="c_bm")
                h_new = stp.tile([2, H], F32, tag="h_bm")
                lstm_pointwise(gps, c_bm, c_new, h_new)
                c_bm, h_bm = c_new, h_new
                if not fused:
                    nc.sync.dma_start(hs[t, :, :], h_new)
                hcT = stp.tile([128, 8, 2], F32R, tag="hcT")
                tph = pt.tile([128, 16], F32, tag="tp")
                for k, (r0, r1) in enumerate(HCH):
                    nc.tensor.transpose(tph[:, 2 * k : 2 * k + 2],
                                        h_new[:, r0:r1], ident[:2, :2])
                    nc.tensor.transpose(tph[:, 8 + 2 * k : 8 + 2 * k + 2],
                                        c_new[:, r0:r1], ident[:2, :2])
                nc.vector.tensor_copy(hcT.bitcast(F32),
                                      tph.rearrange("p (k b) -> p k b", b=2))
                if fused:
                    nc.scalar.copy(
                        hsT_acc.rearrange("p k (b t) -> p k t b",
                                          t=steps)[:, :, t, :],
                        tph.rearrange("p (k b) -> p k b", b=2)[:, 0:4, :])

                tc.strict_bb_all_engine_barrier()
                # ---- mem blend: mem += gb * (cand - mem) ----
                for j, (r0, r1) in enumerate(ECH):
                    gb = pb.tile([128, MEMC], F32, tag="sm")
                    nc.tensor.matmul(gb[: r1 - r0, :], ones2[:, : r1 - r0],
                                     g_sb.bitcast(F32R), start=True, stop=True)
                    dd = rp.tile([128, MEMC], F32, tag="dd")
                    for b in range(2):
                        bc = slice(120 * b, 120 * b + 120)
                        nc.vector.tensor_scalar_add(
                            dd[: r1 - r0, bc],
                            candp[: r1 - r0, j, bc],
                            UT_sb[: r1 - r0, j, 2 * t + b : 2 * t + b + 1])
                    nc.vector.tensor_sub(dd[: r1 - r0, 0:240],
                                         dd[: r1 - r0, 0:240],
                                         memT[: r1 - r0, j, 0:240])
                    nc.vector.tensor_mul(dd[: r1 - r0, 0:240],
                                         dd[: r1 - r0, 0:240],
                                         gb[: r1 - r0, 0:240])
                    nc.vector.tensor_add(memT[: r1 - r0, j, 0:240],
                                         memT[: r1 - r0, j, 0:240],
                                         dd[: r1 - r0, 0:240])

            rctx.close()
            if fused:
                F16 = mybir.dt.float16
                LTOK = BL * steps          # local token cols (128)
                dpool = ctx.enter_context(
                    tc.tile_pool(name="dramp", bufs=1, space="DRAM"))
                hs_locT = dpool.tile([H, LTOK], F16, tag="hs_locT")
                hs_allT = dpool.tile([NC * H, LTOK], F16, tag="hs_allT")
                nc.gpsimd.dma_start(
                    hs_locT.rearrange("(k p) n -> p k n", p=128), hsT_acc)
                nc.gpsimd.collective_compute(
                    "AllGather", mybir.AluOpType.bypass,
                    replica_groups=[list(range(NC))],
                    ins=[hs_locT.opt()], outs=[hs_allT.opt()])

                vw = ctx.enter_context(tc.tile_pool(name="vw", bufs=1))
                vo = ctx.enter_context(tc.tile_pool(name="vo", bufs=3))
                vp = ctx.enter_context(
                    tc.tile_pool(name="vp", bufs=2, space="PSUM"))
                wT = vw.tile([128, 5, VS], F16, tag="wT")
                nc.sync.dma_start(
                    wT, d["woT"][0:640, :].rearrange("(c p) n -> p c n", p=128))
                hT = vw.tile([128, 5, B * L], F16, tag="hT")
                nc.vector.memset(hT[0:1, 4, :], 1.0)
                for c in range(NC):
                    nc.gpsimd.dma_start(
                        hT[:, 0:4, LTOK * c : LTOK * (c + 1)],
                        hs_allT[H * c : H * (c + 1), :].rearrange(
                            "(k p) n -> p k n", p=128))
                chunks = [(o, min(512, VS - o)) for o in range(0, VS, 512)]
                groups = [chunks[i : i + 4] for i in range(0, len(chunks), 4)]
                NBG = 2048
                for mb in range(B * L // 128):
                    for gi, grp in enumerate(groups):
                        g0 = grp[0][0]
                        gw = grp[-1][0] + grp[-1][1] - g0
                        ps = vp.tile([128, NBG], F32, tag="ps")
                        for k in range(5):
                            kw = 128 if k < 4 else 1
                            for (o, w_) in grp:
                                nc.tensor.matmul(
                                    ps[:, o - g0 : o - g0 + w_],
                                    hT[:kw, k, 128 * mb : 128 * mb + 128],
                                    wT[:kw, k, o : o + w_],
                                    start=(k == 0), stop=(k == 4))
                        amax = vo.tile([128, 1], F32, tag="amax")
                        nc.vector.tensor_reduce(
                            op=mybir.AluOpType.abs_max, out=amax,
                            in_=ps[:, :gw], axis=mybir.AxisListType.X)
                        inv = vo.tile([128, 1], F32, tag="inv")
                        nc.vector.reciprocal(inv, amax)
                        fac = vo.tile([128, 1], F32, tag="fac")
                        nc.scalar.activation(fac, inv, AF.Copy, scale=127.0)
                        ot = vo.tile([128, NBG], mybir.dt.int8, tag="ot")
                        nc.scalar.activation(ot[:, :gw], ps[:, :gw], AF.Copy,
                                             scale=fac)
                        nc.sync.dma_start(
                            lg[128 * mb : 128 * mb + 128, g0 : g0 + gw],
                            ot[:, :gw])
                        nc.sync.dma_start(
                            lsc[128 * mb : 128 * mb + 128, gi : gi + 1], amax)
    return nc


def build_k2():
    F16 = mybir.dt.float16
    nc = bass.Bass(trn_type="TRN2", name="cteg_logits")
    hsT = nc.dram_tensor("hsT", [640, B * L], F32, kind="ExternalInput")
    woT = nc.dram_tensor("woT", [640, VS], F32, kind="ExternalInput")
    out = nc.dram_tensor("lg", [B * L, VS], F16, kind="ExternalOutput")
    NBG = 2048
    with tile.TileContext(nc) as tc:
        with ExitStack() as ctx:
            wpo = ctx.enter_context(tc.tile_pool(name="w", bufs=1))
            op = ctx.enter_context(tc.tile_pool(name="o", bufs=3))
            pp = ctx.enter_context(tc.tile_pool(name="p", bufs=2, space="PSUM"))
            hT = wpo.tile([128, 5, B * L], F32R, tag="hT")
            nc.sync.dma_start(
                hT, hsT[0:640, :].rearrange("(c p) n -> p c n", p=128).bitcast(F32R))
            wT = wpo.tile([128, 5, VS], F32R, tag="wT")
            nc.sync.dma_start(
                wT, woT[0:640, :].rearrange("(c p) n -> p c n", p=128).bitcast(F32R))
            chunks = [(o, min(512, VS - o)) for o in range(0, VS, 512)]
            groups = [chunks[i : i + 4] for i in range(0, len(chunks), 4)]
            for mb in range(B * L // 128):
                for grp in groups:
                    g0 = grp[0][0]
                    gw = grp[-1][0] + grp[-1][1] - g0
                    ps = pp.tile([128, NBG], F32, tag="ps")
                    for k in range(5):
                        kw = 128 if k < 4 else 1
                        for (o, w_) in grp:
                            nc.tensor.matmul(
                                ps[:, o - g0 : o - g0 + w_],
                                hT[:kw, k, 128 * mb : 128 * mb + 128],
                                wT[:kw, k, o : o + w_],
                                start=(k == 0), stop=(k == 4))
                    ot = op.tile([128, NBG], F16, tag="ot")
                    nc.scalar.copy(ot[:, :gw], ps[:, :gw])
                    nc.sync.dma_start(
                        out[128 * mb : 128 * mb + 128, g0 : g0 + gw],
                        ot[:, :gw])
    return nc


K1_WEIGHT_KEYS = (
    "enc_Wih_f", "enc_b_f", "enc_Wih_b", "enc_b_b", "enc_Whh_f", "enc_Whh_b",
    "dec_Wih", "dec_b", "dec_Whh", "Wp1", "bp1", "Wp2", "bp2", "Wep", "bep",
    "Wi1", "bi1", "Wmp", "bmp", "attn_v")


def _prep_shared(inputs):
    """Replicated k1 weight tensors (host layout/padding)."""
    f = lambda x: np.ascontiguousarray(np.asarray(x), dtype=np.float32)
    wih = f(inputs["dec_Wih"])
    shared = {
        "enc_xT_f": np.vstack([f(inputs["enc_Wih_f"]).T, f(inputs["enc_b_f"])[None]]),
        "enc_xT_b": np.vstack([f(inputs["enc_Wih_b"]).T, f(inputs["enc_b_b"])[None]]),
        "enc_hT_f": f(inputs["enc_Whh_f"]).T.copy(),
        "enc_hT_b": f(inputs["enc_Whh_b"]).T.copy(),
        "decXT": np.vstack([wih[:, :E].T, f(inputs["dec_b"])[None]]),
        "decAT": wih[:, E : E + H].T.copy(),
        "decMT": wih[:, E + H :].T.copy(),
        "decHT": f(inputs["dec_Whh"]).T.copy(),
        "wp1T_a": np.vstack([f(inputs["Wp1"]).T, f(inputs["bp1"])[None]]),
        "wp2T_a": np.vstack([f(inputs["Wp2"]).T, f(inputs["bp2"])[None]]),
        "wepT_a": np.vstack([f(inputs["Wep"]).T, f(inputs["bep"])[None]]),
        "wi1T_a": np.vstack([f(inputs["Wi1"]).T,
                             (f(inputs["bi1"]) + f(inputs["bmp"]))[None]]),
        "wmpT": f(inputs["Wmp"]).T.copy(),
        "attn_vT": f(inputs["attn_v"])[:, None].copy(),
    }
    mask_attn = np.zeros((2 * T, BL), np.float32)
    for t in range(T):
        for b in range(BL):
            mask_attn[2 * t + b, b] = 1.0
    shared["mask_attn"] = mask_attn
    mask_memT = np.zeros((BL, MEMC), np.float32)
    for b in range(BL):
        mask_memT[b, 120 * b : 120 * (b + 1)] = 1.0
    shared["mask_memT"] = mask_memT
    pad_to = {"enc_xT_f": 384, "enc_xT_b": 384, "decXT": 384, "decMT": 384,
              "wp1T_a": 640, "wp2T_a": 640, "wepT_a": 640, "wi1T_a": 384,
              "wmpT": 384}
    for k, rows in pad_to.items():
        v = shared[k]
        shared[k] = np.pad(v, ((0, rows - v.shape[0]), (0, 0)))
    return {k: np.ascontiguousarray(v, np.float32) for k, v in shared.items()}


def _prep_data(inputs):
    """Per-core embedding-gathered activations, concat over cores on axis 0."""
    emb = np.ascontiguousarray(np.asarray(inputs["embedding"]), np.float32)
    topic = np.asarray(inputs["topic"]).astype(np.int64)
    essay = np.asarray(inputs["essay_input"]).astype(np.int64)
    mems = np.asarray(inputs["mems"]).astype(np.int64)
    te = emb[topic]          # [B, T, E]
    ee = emb[essay]          # [B, L, E]
    me = emb[mems]           # [B, M, E]

    topicT = np.zeros((NC, 384, 2 * T), np.float32)
    essayT = np.zeros((NC, 384, 2 * L), np.float32)
    memT0 = np.zeros((NC, 384, MEMC), np.float32)
    topicT[:, E] = 1.0
    essayT[:, E] = 1.0
    # [B,S,E] -> per-core [E, 2*S] with (t,b) interleave on cols
    topicT[:, :E] = np.moveaxis(
        te.reshape(NC, BL, T, E), (1, 2, 3), (3, 2, 1)).reshape(NC, E, 2 * T)
    essayT[:, :E] = np.moveaxis(
        ee.reshape(NC, BL, L, E), (1, 2, 3), (3, 2, 1)).reshape(NC, E, 2 * L)
    memT0[:, :E, : 2 * M] = np.moveaxis(
        me.reshape(NC, BL, M, E), (1, 2, 3), (2, 3, 1)).reshape(NC, E, 2 * M)
    return {
        "topicT_a": np.ascontiguousarray(topicT).reshape(NC * 384, 2 * T),
        "essayT_a": np.ascontiguousarray(essayT).reshape(NC * 384, 2 * L),
        "memT0": np.ascontiguousarray(memT0).reshape(NC * 384, MEMC),
    }


def _prep_wout(inputs, dtype=np.float16):
    """Vocab-sharded transposed output projection, concat over cores."""
    wo = np.asarray(inputs["Wout"], np.float32)
    bo = np.asarray(inputs["bout"], np.float32)
    woT = np.zeros((NC, 640, VS), dtype)
    woT[:, :H] = wo.reshape(NC, VS, H).transpose(0, 2, 1)
    woT[:, H] = bo.reshape(NC, VS)
    return np.ascontiguousarray(woT).reshape(NC * 640, VS)


def _split_multi_waits(bir_json):
    """walrus in this env accepts at most ONE sync wait per instruction
    (S3_LW/CTRL_NO etc. reject more). Hoist extra waits onto same-engine
    NoOps inserted immediately before the instruction — sequencers execute
    in order, so the happens-before relation is preserved."""
    import json

    d = json.loads(bir_json)
    cnt = [0]
    for f in d["functions"]:
        for bb in f["blocks"]:
            out = []
            for inst in bb["instructions"]:
                si = inst.get("sync_info") or {}
                waits = si.get("on_wait") or []
                if len(waits) > 1 and inst["opcode"] != "ISA":
                    for w in waits[:-1]:
                        cnt[0] += 1
                        out.append({
                            "debug": inst.get("debug", 0),
                            "engine": inst["engine"],
                            "ins": [],
                            "outs": [],
                            "name": f"{inst['name']}-w{cnt[0]}",
                            "opcode": "NoOp",
                            "sync_info": {"on_update": [], "on_wait": [w]},
                        })
                    si["on_wait"] = [waits[-1]]
                    inst["sync_info"] = si
                out.append(inst)
            bb["instructions"] = out
    return json.dumps(d).encode()


def _patch_compile():
    import concourse.bass_utils as bu
    import concourse.bass2jax as b2j
    if getattr(bu, "_wait_patched", False):
        return
    orig = bu.compile_bir_kernel

    def patched(bir_json, tmpdir, neff_name="file.neff"):
        return orig(_split_multi_waits(bir_json), tmpdir, neff_name)

    bu.compile_bir_kernel = patched
    b2j.compile_bir_kernel = patched
    bu._wait_patched = True


# ---------------- persistent runner ----------------

import zlib


def _crc(*arrs):
    h = 0
    for a in arrs:
        a = np.ascontiguousarray(np.asarray(a))
        h = zlib.crc32(a.view(np.uint8).reshape(-1), h)
    return h


def _mesh():
    import jax
    from jax.sharding import Mesh
    if "mesh" not in _cache:
        devs = jax.devices()[:NC]
        assert len(devs) == NC
        _cache["mesh"] = Mesh(np.asarray(devs), ("core",))
    return _cache["mesh"]


def _meta(nc_obj):
    import jax
    partition_name = (nc_obj.partition_id_tensor.name
                      if nc_obj.partition_id_tensor else None)
    in_names, out_names, out_avals = [], [], []
    for alloc in nc_obj.m.functions[0].allocations:
        if not isinstance(alloc, mybir.MemoryLocationSet):
            continue
        name = alloc.memorylocations[0].name
        if alloc.kind == "ExternalInput":
            if name != partition_name:
                in_names.append(name)
        elif alloc.kind == "ExternalOutput":
            out_names.append(name)
            out_avals.append(jax.core.ShapedArray(
                tuple(alloc.tensor_shape), mybir.dt.np(alloc.dtype)))
    return in_names, out_names, out_avals, partition_name


def _make_fn(nc_obj, core_sharded_names):
    """Jitted SPMD launcher for a finalized bass module. Outputs are fully
    written by our kernels, so no donated zero buffers are passed."""
    import jax
    from jax.experimental.shard_map import shard_map
    from jax.sharding import PartitionSpec as P
    from concourse.bass2jax import _bass_exec_p, partition_id_tensor

    in_names, out_names, out_avals, partition_name = _meta(nc_obj)
    bind_names = tuple(in_names) + ((partition_name,) if partition_name else ())

    def _body(*args):
        operands = list(args)
        if partition_name:
            operands.append(partition_id_tensor())
        outs = _bass_exec_p.bind(
            *operands, out_avals=tuple(out_avals), in_names=bind_names,
            out_names=tuple(out_names), lowering_input_output_aliases=(),
            sim_require_finite=True, sim_require_nnan=True, nc=nc_obj)
        return tuple(outs)

    mesh = _mesh()
    in_specs = tuple(P("core") if n in core_sharded_names else P()
                     for n in in_names)
    out_specs = (P("core"),) * len(out_names)
    fn = jax.jit(shard_map(_body, mesh=mesh, in_specs=in_specs,
                           out_specs=out_specs, check_rep=False))
    return fn, in_names, out_names


def _upload_rep(np_map):
    """Host -> dev0 (1x over the wire) -> all-device replicate (D2D)."""
    import jax
    from jax.sharding import NamedSharding, PartitionSpec as P
    mesh = _mesh()
    vals = list(np_map.values())
    on0 = jax.device_put(vals, jax.devices()[0])
    jax.block_until_ready(on0)
    rep = jax.device_put(on0, NamedSharding(mesh, P()))
    jax.block_until_ready(rep)
    return dict(zip(np_map.keys(), rep))


def _upload_shard(np_list):
    import jax
    from jax.sharding import NamedSharding, PartitionSpec as P
    s = NamedSharding(_mesh(), P("core"))
    out = jax.device_put(np_list, s)
    jax.block_until_ready(out)
    return out


def _get_gather_fn():
    """hs [NC*L, BL, H] sharded-by-core -> hsT [640, B*L] replicated."""
    import jax
    import jax.numpy as jnp
    from functools import partial
    from jax.sharding import NamedSharding, PartitionSpec as P
    if "gather_fn" in _cache:
        return _cache["gather_fn"]
    s_rep = NamedSharding(_mesh(), P())

    @partial(jax.jit, out_shardings=s_rep)
    def g(hs):
        x = hs.reshape(NC, L, BL, H).transpose(0, 2, 1, 3).reshape(B * L, H)
        hT = x.T
        ones = jnp.ones((1, B * L), jnp.float32)
        pad = jnp.zeros((640 - H - 1, B * L), jnp.float32)
        return jnp.concatenate([hT, ones, pad], axis=0)

    _cache["gather_fn"] = g
    return g


def _upload_weights(inputs, wo_dtype=np.float16):
    k1key = _crc(*(inputs[k] for k in K1_WEIGHT_KEYS))
    if _cache.get("k1key") != k1key:
        _cache["k1w"] = _upload_rep(_prep_shared(inputs))
        _cache["k1key"] = k1key
    k2key = (_crc(inputs["Wout"], inputs["bout"]), np.dtype(wo_dtype).str)
    if _cache.get("k2key") != k2key:
        _cache["woT"] = _upload_shard([_prep_wout(inputs, wo_dtype)])[0]
        _cache["k2key"] = k2key


def _assemble(lg):
    lg_np = np.asarray(lg).reshape(NC, B * L, VS)       # float16
    out = np.empty((B * L, V), np.float32)
    for c in range(NC):
        out[:, VS * c : VS * (c + 1)] = lg_np[c]
    return out.reshape(B, L, V)


def _kernel_fused(inputs):
    if "kf_fn" not in _cache:
        _cache["kf"] = build_k1(fused=True)
        _cache["kf_fn"] = _make_fn(
            _cache["kf"], {"topicT_a", "essayT_a", "memT0", "woT"})
    _upload_weights(inputs, np.float16)
    data = _prep_data(inputs)
    ddev = dict(zip(data.keys(), _upload_shard(list(data.values()))))
    fn, innames, _ = _cache["kf_fn"]
    ops = [ddev[n] if n in ddev
           else (_cache["woT"] if n == "woT" else _cache["k1w"][n])
           for n in innames]
    lg, lsc = fn(*ops)
    lg_np = np.asarray(lg).reshape(NC, B * L, VS)       # int8
    sc_np = np.asarray(lsc).reshape(NC, B * L, 2).astype(np.float32) / 127.0
    out = np.empty((B * L, V), np.float32)
    for c in range(NC):
        for gi, (g0, g1) in enumerate(((0, 2048), (2048, VS))):
            np.multiply(lg_np[c][:, g0:g1], sc_np[c][:, gi : gi + 1],
                        out=out[:, VS * c + g0 : VS * c + g1])
    return out.reshape(B, L, V)


def _kernel_split(inputs):
    if "k1_fn" not in _cache:
        _cache["k1"] = build_k1()
        _cache["k1_fn"] = _make_fn(
            _cache["k1"], {"topicT_a", "essayT_a", "memT0"})
    if "k2_fn" not in _cache:
        _cache["k2"] = build_k2()
        _cache["k2_fn"] = _make_fn(_cache["k2"], {"woT"})
    _upload_weights(inputs, np.float32)
    data = _prep_data(inputs)
    ddev = dict(zip(data.keys(), _upload_shard(list(data.values()))))

    fn1, in1, _ = _cache["k1_fn"]
    ops1 = [ddev[n] if n in ddev else _cache["k1w"][n] for n in in1]
    (hs,) = fn1(*ops1)

    hsT = _get_gather_fn()(hs)

    fn2, in2, _ = _cache["k2_fn"]
    ops2 = [hsT if n == "hsT" else _cache["woT"] for n in in2]
    (lg,) = fn2(*ops2)
    return _assemble(lg)


def kernel(**inputs):
    _patch_compile()
    from concourse.bass2jax import install_neuronx_cc_hook
    install_neuronx_cc_hook()

    if not _cache.get("fused_broken"):
        try:
            return _kernel_fused(inputs)
        except Exception:
            import traceback
            traceback.print_exc()
            _cache["fused_broken"] = True
    return _kernel_split(inputs)



# revision 25
# speedup vs baseline: 1.6095x; 1.6095x over previous
"""CTEG kernel for 8x TRN2 NeuronCores.

K1 (SPMD, 8 cores): data-parallel recurrence (2 batch rows/core): encoder
   (bi-LSTM over T=8) + 64-step decoder with memory network + attention,
   emitting decoder hidden states hs [64, 2, 512].
K2 (SPMD, 8 cores): vocab-sharded projection: each core computes
   logits[:, :, c*4000:(c+1)*4000] = hs_all @ Wout_c.T + bout_c.

Host side: embedding gathers, weight transposes, shard assembly.
"""

import sys

sys.path.insert(0, "/opt/trn_rl_repo")

from contextlib import ExitStack

import numpy as np

import concourse.bass as bass
import concourse.mybir as mybir
import concourse.tile as tile
from concourse.masks import make_identity

B, T, L, V, E, H, A, M = 16, 8, 64, 32000, 300, 512, 128, 120
NC = 8
BL = B // NC          # 2 batch rows per core
VS = V // NC          # 4000 vocab rows per core
F32 = mybir.dt.float32
F32R = mybir.dt.float32  # fp32r needs rounded producers; plain fp32 for now
AF = mybir.ActivationFunctionType
MEMC = 256            # B*M=240 padded to 256 (fp32r needs free>=256 for 1cyc/row)
ECH = [(0, 128), (128, 256), (256, 300)]             # E row chunks
EACH = [(0, 128), (128, 256), (256, 301)]            # E+1 (bias row) chunks
HCH = [(0, 128), (128, 256), (256, 384), (384, 512)]

_cache = {}


def _chunked_load(nc, pool, dram, chunks, ncols, tag, dtype=F32R):
    # dram is padded to len(chunks)*128 rows; single DMA, chunk-major layout
    nch = len(chunks)
    t_ = pool.tile([128, nch, ncols], dtype, tag=tag)
    src = dram[0 : 128 * nch, :].rearrange("(c p) n -> p c n", p=128)
    if dtype == F32R:
        src = src.bitcast(F32R)
    nc.sync.dma_start(t_, src)
    return t_


def build_k1(steps=L, tsteps=T, fused=False):
    nc = bass.Bass(trn_type="TRN2", name="cteg_fused" if fused else "cteg_rec",
                   num_devices=NC if fused else None)
    d = {}

    def inp(name, shape):
        d[name] = nc.dram_tensor(name, list(shape), F32, kind="ExternalInput")
        return d[name]

    TB = 2 * tsteps
    inp("topicT_a", (384, TB))
    inp("essayT_a", (384, 2 * steps))
    inp("memT0", (384, MEMC))
    inp("enc_xT_f", (384, 4 * H))
    inp("enc_xT_b", (384, 4 * H))
    inp("enc_hT_f", (H, 4 * H))
    inp("enc_hT_b", (H, 4 * H))
    inp("decXT", (384, 4 * H))
    inp("decHT", (H, 4 * H))
    inp("decMT", (384, 4 * H))
    inp("decAT", (H, 4 * H))
    inp("wp1T_a", (640, E))
    inp("wp2T_a", (640, A))
    inp("wepT_a", (640, A))
    inp("wi1T_a", (384, E))
    inp("wmpT", (384, E))
    inp("attn_vT", (A, 1))
    inp("mask_attn", (TB, BL))      # [(t,b), b'] = (b==b')
    inp("mask_memT", (BL, MEMC))    # [b', c] = (c//120==b'), pad cols 0
    if fused:
        d["woT"] = nc.dram_tensor("woT", [640, VS], mybir.dt.float16,
                                  kind="ExternalInput")
        # int8 logits + per-(row, col-group) absmax scales; host dequantizes
        lg = nc.dram_tensor("lg", [B * L, VS], mybir.dt.int8,
                            kind="ExternalOutput")
        lsc = nc.dram_tensor("lsc", [B * L, 2], F32, kind="ExternalOutput")
    else:
        hs = nc.dram_tensor("hs", [steps, BL, H], F32, kind="ExternalOutput")

    with tile.TileContext(nc) as tc:
        with ExitStack() as ctx:
            wp = ctx.enter_context(tc.tile_pool(name="wts", bufs=1))
            sp = ctx.enter_context(tc.tile_pool(name="big", bufs=1))
            stp = ctx.enter_context(tc.tile_pool(name="state", bufs=3))
            rp = ctx.enter_context(tc.tile_pool(name="roll", bufs=4))
            sgp = ctx.enter_context(tc.tile_pool(name="sigp", bufs=2))
            # recurrence-phase pools (PSUM + decoder weights); closed before
            # the fused vocab-projection phase to free PSUM banks and SBUF
            rctx = ExitStack()
            pg = rctx.enter_context(tc.tile_pool(name="psg", bufs=1, space="PSUM"))
            pb = rctx.enter_context(tc.tile_pool(name="psb", bufs=1, space="PSUM"))
            pt = rctx.enter_context(tc.tile_pool(name="pst", bufs=1, space="PSUM"))

            # ---- small resident constants ----
            topicT = _chunked_load(nc, wp, d["topicT_a"], EACH, TB, "topicT")
            essayT = _chunked_load(nc, wp, d["essayT_a"], EACH, 2 * steps, "essayT")
            HACH = [(0, 128), (128, 256), (256, 384), (384, 512), (512, 513)]
            wp1T = _chunked_load(nc, wp, d["wp1T_a"], HACH, E, "wp1T")
            wp2T = _chunked_load(nc, wp, d["wp2T_a"], HACH, A, "wp2T")
            wepT = _chunked_load(nc, wp, d["wepT_a"], HACH, A, "wepT")
            wi1T = _chunked_load(nc, wp, d["wi1T_a"], EACH, E, "wi1T")
            wmpT = _chunked_load(nc, wp, d["wmpT"], ECH, E, "wmpT")
            attn_vT = wp.tile([A, 1], F32R, tag="attn_vT")
            nc.sync.dma_start(attn_vT, d["attn_vT"][:, :].bitcast(F32R))
            mask_attn = wp.tile([TB, BL], F32, tag="mask_attn")
            nc.sync.dma_start(mask_attn, d["mask_attn"][:, :])
            mask_memT = wp.tile([BL, MEMC], F32, tag="mask_memT")
            nc.sync.dma_start(mask_memT, d["mask_memT"][:, :])
            mask_memTr = mask_memT.bitcast(F32R)

            ident = wp.tile([128, 128], F32, tag="ident")
            make_identity(nc, ident)
            identr = ident.bitcast(F32R)
            ones2f = wp.tile([2, 128], F32, tag="ones2")
            nc.vector.memset(ones2f, 1.0)
            ones2 = ones2f.bitcast(F32R)

            memT = sp.tile([128, 3, MEMC], F32, tag="memT")
            nc.sync.dma_start(
                memT, d["memT0"][0:384, :].rearrange("(c p) n -> p c n", p=128))
            memTr = memT.bitcast(F32R)

            h_bm = stp.tile([2, H], F32, tag="h_bm")
            c_bm = stp.tile([2, H], F32, tag="c_bm")
            # enc_outs stored transposed: eoT[:, k, 2t+b] = enc_outs[b, t, 128k+p]
            eoT = sp.tile([128, 4, TB], F32, tag="eoT")
            if fused:
                # hsT_acc[p, k, b*steps+t] = dec h_t[b, 128k+p] (f16 for the
                # fp16 vocab projection; recurrence itself stays fp32)
                hsT_acc = sp.tile([128, 4, BL * steps], mybir.dt.float16,
                                  tag="hsT_acc")

            def lstm_pointwise(gate_ps, cprev, cnext, hnext):
                # gate_ps [2, 4H] flat: i|f|g|o
                sig = sgp.tile([2, 4 * H], F32, tag="sig")
                nc.scalar.activation(sig[:, 0 : 2 * H], gate_ps[:, 0 : 2 * H],
                                     AF.Sigmoid)
                nc.scalar.activation(sig[:, 2 * H : 3 * H],
                                     gate_ps[:, 2 * H : 3 * H], AF.Tanh)
                nc.scalar.activation(sig[:, 3 * H : 4 * H],
                                     gate_ps[:, 3 * H : 4 * H], AF.Sigmoid)
                tmp = rp.tile([2, H], F32, tag="ctmp")
                nc.vector.tensor_mul(cnext, sig[:, H : 2 * H], cprev)
                nc.vector.tensor_mul(tmp, sig[:, 0:H], sig[:, 2 * H : 3 * H])
                nc.vector.tensor_add(cnext, cnext, tmp)
                tc2 = rp.tile([2, H], F32, tag="tc2")
                nc.scalar.activation(tc2, cnext, AF.Tanh)
                nc.vector.tensor_mul(hnext, sig[:, 3 * H : 4 * H], tc2)

            # ================= ENCODER =================
            hfin = {}
            cfin = {}
            with ExitStack() as ectx:
                eps2 = ectx.enter_context(tc.tile_pool(name="encs", bufs=4))
                for dr in ("f", "b"):
                    with ExitStack() as dctx:
                        epd = dctx.enter_context(
                            tc.tile_pool(name=f"encw{dr}", bufs=1))
                        xsb = epd.tile([TB, 4 * H], F32R, tag="xsb")
                        with ExitStack() as xctx:
                            xp = xctx.enter_context(
                                tc.tile_pool(name=f"encx{dr}", bufs=1))
                            xpp = xctx.enter_context(
                                tc.tile_pool(name=f"encxp{dr}", bufs=1,
                                             space="PSUM"))
                            ew = _chunked_load(nc, xp, d[f"enc_xT_{dr}"], EACH,
                                               4 * H, "ew")
                            for hf_ in range(2):
                                xps = xpp.tile([TB, 2 * H], F32, tag="xps")
                                for ki, (r0, r1) in enumerate(EACH):
                                    for c2 in range(2):
                                        cc = 2 * hf_ + c2
                                        nc.tensor.matmul(
                                            xps[:, 512 * c2 : 512 * c2 + 512],
                                            topicT[: r1 - r0, ki, :],
                                            ew[: r1 - r0, ki,
                                               512 * cc : 512 * cc + 512],
                                            start=(ki == 0), stop=(ki == 2))
                                nc.scalar.copy(
                                    xsb.bitcast(F32)[:, 1024 * hf_ :
                                                     1024 * hf_ + 1024], xps)
                        ehw = _chunked_load(
                            nc, epd, d[f"enc_hT_{dr}"],
                            [(128 * k, 128 * k + 128) for k in range(4)],
                            4 * H, "ehw")
                        hT0 = eps2.tile([128, 4, 2], F32R, tag="ehT")
                        nc.vector.memset(hT0.bitcast(F32), 0.0)
                        hT = None
                        cd = eps2.tile([2, H], F32, tag="ecd")
                        nc.vector.memset(cd, 0.0)
                        for s in range(tsteps):
                            t = s if dr == "f" else tsteps - 1 - s
                            tc.strict_bb_all_engine_barrier()
                            gps = pg.tile([2, 4 * H], F32, tag="gps")
                            if s == 0:
                                hT_prev = hT0
                            elif dr == "f":
                                hT_prev = eoT.bitcast(F32R)[
                                    :, :, 2 * (t - 1) : 2 * (t - 1) + 2]
                            else:
                                hT_prev = hT
                            for cc in range(4):
                                cs = slice(512 * cc, 512 * cc + 512)
                                for ki in range(4):
                                    nc.tensor.matmul(
                                        gps[:, cs],
                                        hT_prev[:, ki, :], ehw[:, ki, cs],
                                        start=(ki == 0), stop=False)
                                nc.tensor.matmul(
                                    gps[:, cs],
                                    identr[:TB, 2 * t : 2 * t + 2],
                                    xsb[:, cs],
                                    start=False, stop=True)
                            cnew = eps2.tile([2, H], F32, tag="ecn")
                            hnew = eps2.tile([2, H], F32, tag="ehn")
                            lstm_pointwise(gps, cd, cnew, hnew)
                            cd = cnew
                            tp = pt.tile([128, 8], F32, tag="tp")
                            for k, (r0, r1) in enumerate(HCH):
                                nc.tensor.transpose(
                                    tp[:, 2 * k : 2 * k + 2],
                                    hnew[:, r0:r1], ident[:2, :2])
                            tdst = eoT[:, :, 2 * t : 2 * t + 2]
                            tsrc = tp.rearrange("p (k b) -> p k b", b=2)
                            if dr == "f":
                                nc.vector.tensor_copy(tdst, tsrc)
                            else:
                                nc.vector.tensor_add(tdst, tdst, tsrc)
                            if s < tsteps - 1:
                                if dr == "f":
                                    hT = None  # fwd reads eoT directly
                                else:
                                    hT = eps2.tile([128, 4, 2], F32R, tag="ehT")
                                    nc.vector.tensor_copy(hT.bitcast(F32), tsrc)
                            else:
                                hfin[dr] = hnew
                        cfin[dr] = cd
                nc.vector.tensor_add(h_bm, hfin["f"], hfin["b"])
                nc.vector.tensor_add(c_bm, cfin["f"], cfin["b"])

            # dec weights in a pool opened after encoder pools closed
            H4CH = [(128 * k, 128 * k + 128) for k in range(4)]
            dwp = rctx.enter_context(tc.tile_pool(name="decw", bufs=1))
            decXT = _chunked_load(nc, dwp, d["decXT"], EACH, 4 * H, "decXT")
            decHT = _chunked_load(nc, dwp, d["decHT"], H4CH, 4 * H, "decHT")
            decMT = _chunked_load(nc, dwp, d["decMT"], ECH, 4 * H, "decMT")

            # hcT: chunks 0-3 = hT, 4-7 = cT
            hcT = stp.tile([128, 8, 2], F32R, tag="hcT")
            tp0 = pt.tile([128, 16], F32, tag="tp")
            for k, (r0, r1) in enumerate(HCH):
                nc.tensor.transpose(tp0[:, 2 * k : 2 * k + 2], h_bm[:, r0:r1],
                                    ident[:2, :2])
                nc.tensor.transpose(tp0[:, 8 + 2 * k : 8 + 2 * k + 2],
                                    c_bm[:, r0:r1], ident[:2, :2])
            nc.vector.tensor_copy(hcT.bitcast(F32),
                                  tp0.rearrange("p (k b) -> p k b", b=2))

            tc.strict_bb_all_engine_barrier()
            # ---- precompute phase ----
            TBL = 2 * steps
            P_sb = sp.tile([TB, 4 * H], F32R, tag="P_sb")
            epT_sb = sp.tile([A, TB], F32, tag="epT_sb")
            UT_sb = sp.tile([128, 3, TBL], F32, tag="UT_sb")
            XD_sb = sp.tile([TBL, 4 * H], F32R, tag="XD_sb")
            with ExitStack() as pctx:
                ppre = pctx.enter_context(
                    tc.tile_pool(name="pre", bufs=1, space="PSUM"))
                dap = pctx.enter_context(tc.tile_pool(name="decA", bufs=1))
                for hf_ in range(2):
                    decAT = dap.tile([128, 4, 1024], F32R, tag="decAT")
                    nc.sync.dma_start(
                        decAT,
                        d["decAT"][:, 1024 * hf_ : 1024 * hf_ + 1024].rearrange(
                            "(c p) n -> p c n", p=128).bitcast(F32R))
                    pps = ppre.tile([TB, 2 * H], F32, tag="pre")
                    for ki in range(4):
                        for c2 in range(2):
                            nc.tensor.matmul(
                                pps[:, 512 * c2 : 512 * c2 + 512], eoT.bitcast(F32R)[:, ki, :],
                                decAT[:, ki, 512 * c2 : 512 * c2 + 512],
                                start=(ki == 0), stop=(ki == 3))
                    nc.scalar.copy(
                        P_sb.bitcast(F32)[:, 1024 * hf_ : 1024 * hf_ + 1024], pps)

                # enc_procT [A, TB] (A-major): lhsT = wepT chunks, rhs = eoT (+ones)
                eph = ppre.tile([A, TB], F32, tag="pre")
                for ki in range(4):
                    nc.tensor.matmul(eph, wepT[:, ki, :], eoT.bitcast(F32R)[:, ki, :],
                                     start=(ki == 0), stop=False)
                nc.tensor.matmul(eph, wepT[0:1, 4, :], ones2[0:1, 0:TB],
                                 start=False, stop=True)
                nc.vector.tensor_copy(epT_sb, eph)

                for j, (c0, c1) in enumerate(ECH):
                    ups = ppre.tile([128, TBL], F32, tag="pre")
                    for ki, (r0, r1) in enumerate(EACH):
                        nc.tensor.matmul(ups[: c1 - c0, :],
                                         wi1T[: r1 - r0, ki, c0:c1],
                                         essayT[: r1 - r0, ki, :],
                                         start=(ki == 0), stop=(ki == 2))
                    nc.scalar.copy(UT_sb[: c1 - c0, j, :], ups[: c1 - c0, :])

                for cc in range(4):
                    xps2 = ppre.tile([TBL, H], F32, tag="pre")
                    for ki, (r0, r1) in enumerate(EACH):
                        nc.tensor.matmul(xps2, essayT[: r1 - r0, ki, :],
                                         decXT[: r1 - r0, ki,
                                               512 * cc : 512 * cc + 512],
                                         start=(ki == 0), stop=(ki == 2))
                    nc.scalar.copy(XD_sb.bitcast(F32)[:, 512 * cc : 512 * cc + 512],
                                   xps2)

            pc = rctx.enter_context(tc.tile_pool(name="psc", bufs=1, space="PSUM"))

            # ================= DECODER =================
            for t in range(steps):
                tc.strict_bb_all_engine_barrier()
                # ---- mem write pipeline (h-independent) ----
                candp = pc.tile([128, 3, MEMC], F32, tag="candp")
                for j, (c0, c1) in enumerate(ECH):
                    for ki, (r0, r1) in enumerate(ECH):
                        nc.tensor.matmul(candp[: c1 - c0, j, :],
                                         wmpT[: r1 - r0, ki, c0:c1],
                                         memTr[: r1 - r0, ki, :],
                                         start=(ki == 0), stop=(ki == 2))
                gps_m = pb.tile([2, MEMC], F32, tag="sm")
                for ki, (r0, r1) in enumerate(ECH):
                    nc.tensor.matmul(gps_m, essayT[: r1 - r0, ki, 2 * t : 2 * t + 2],
                                     memTr[: r1 - r0, ki, :],
                                     start=(ki == 0), stop=(ki == 2))
                g_sb = rp.tile([2, MEMC], F32, tag="g_sb")
                nc.scalar.activation(g_sb, gps_m, AF.Sigmoid)
                nc.vector.tensor_mul(g_sb, g_sb, mask_memT)

                tc.strict_bb_all_engine_barrier()
                # ---- mem read: v, sim, mt ----
                vps = pb.tile([2, E], F32, tag="sm")
                for ki in range(4):
                    nc.tensor.matmul(vps, hcT[:, ki, :], wp1T[:, ki, :],
                                     start=(ki == 0), stop=False)
                nc.tensor.matmul(vps, ones2[0:1, 0:2], wp1T[0:1, 4, :],
                                 start=False, stop=True)
                v_bm = rp.tile([2, E], F32, tag="v_bm")
                nc.scalar.activation(v_bm, vps, AF.Tanh)
                vT = rp.tile([128, 3, 2], F32R, tag="vT")
                tpv = pt.tile([128, 6], F32, tag="tp")
                for j, (r0, r1) in enumerate(ECH):
                    nc.tensor.transpose(tpv[: r1 - r0, 2 * j : 2 * j + 2],
                                        v_bm[:, r0:r1], ident[:2, :2])
                for j, (r0, r1) in enumerate(ECH):
                    nc.vector.tensor_copy(vT.bitcast(F32)[: r1 - r0, j, :],
                                          tpv[: r1 - r0, 2 * j : 2 * j + 2])
                sps = pb.tile([2, MEMC], F32, tag="sm")
                for ki, (r0, r1) in enumerate(ECH):
                    nc.tensor.matmul(sps, vT[: r1 - r0, ki, :],
                                     memTr[: r1 - r0, ki, :],
                                     start=(ki == 0), stop=(ki == 2))
                es = rp.tile([2, MEMC], F32, tag="es")
                nc.scalar.activation(es, sps, AF.Exp)
                den = rp.tile([2, 1], F32, tag="den")
                nc.vector.tensor_mul(es, es, mask_memT)
                nc.vector.tensor_reduce(op=mybir.AluOpType.add, out=den,
                                        in_=es, axis=mybir.AxisListType.X)
                nc.vector.reciprocal(den, den)
                nc.vector.tensor_scalar_mul(es, es, den)
                esr = es.bitcast(F32R)

                tc.strict_bb_all_engine_barrier()
                mtT = rp.tile([128, 3, 2], F32R, tag="mtT")
                junk = rp.tile([128, 120], F32, tag="junk")
                for j, (r0, r1) in enumerate(ECH):
                    arep = pb.tile([128, MEMC], F32, tag="sm")
                    nc.tensor.matmul(arep[: r1 - r0, :], ones2[:, : r1 - r0], esr,
                                     start=True, stop=True)
                    for b in range(2):
                        nc.vector.tensor_mul(
                            junk[: r1 - r0, :],
                            memT[: r1 - r0, j, 120 * b : 120 * b + 120],
                            arep[: r1 - r0, 120 * b : 120 * b + 120])
                        nc.vector.tensor_reduce(
                            op=mybir.AluOpType.add,
                            out=mtT.bitcast(F32)[: r1 - r0, j, b : b + 1],
                            in_=junk[: r1 - r0, :], axis=mybir.AxisListType.X)

                tc.strict_bb_all_engine_barrier()
                # ---- attention ----
                qps = pb.tile([A, 2], F32, tag="sm")
                for ki in range(4):
                    nc.tensor.matmul(qps, wp2T[:, ki, :], hcT[:, 4 + ki, :],
                                     start=(ki == 0), stop=False)
                nc.tensor.matmul(qps, wp2T[0:1, 4, :], ones2[0:1, 0:2],
                                 start=False, stop=True)
                qsb = rp.tile([A, 2], F32, tag="qsb")
                nc.vector.tensor_copy(qsb, qps)
                tha = rp.tile([A, TB], F32, tag="tha")
                for b in range(2):
                    nc.scalar.activation(
                        tha.rearrange("a (t b) -> a t b", b=2)[:, :, b],
                        epT_sb.rearrange("a (t b) -> a t b", b=2)[:, :, b],
                        AF.Tanh, bias=qsb[:, b : b + 1], scale=1.0)
                scps = pb.tile([1, TB], F32, tag="sm")
                nc.tensor.matmul(scps, attn_vT, tha.bitcast(F32R),
                                 start=True, stop=True)
                esc = rp.tile([1, TB], F32, tag="esc")
                nc.scalar.activation(esc, scps, AF.Exp)
                escT = pt.tile([TB, 1], F32, tag="tp")
                nc.tensor.transpose(escT, esc, ident[0:1, 0:1])
                escTs = rp.tile([TB, 1], F32, tag="escTs")
                nc.vector.tensor_copy(escTs, escT)
                sms = pb.tile([2, 1], F32, tag="sm")
                nc.tensor.matmul(sms, mask_attn.bitcast(F32R),
                                 escTs.bitcast(F32R), start=True, stop=True)
                rden = rp.tile([2, 1], F32, tag="rden")
                nc.vector.reciprocal(rden, sms)
                rrT = pt.tile([1, 2], F32, tag="tp")
                nc.tensor.transpose(rrT, rden, ident[:2, :2])
                rr_sb = rp.tile([1, 2], F32, tag="rr_sb")
                nc.vector.tensor_copy(rr_sb, rrT)
                rrep = pb.tile([TB, 2], F32, tag="sm")
                nc.tensor.matmul(rrep, ones2[0:1, 0:TB], rr_sb.bitcast(F32R),
                                 start=True, stop=True)
                alBD = rp.tile([TB, BL], F32, tag="alBD")
                nc.vector.tensor_scalar_mul(alBD, mask_attn, escTs)
                nc.vector.tensor_mul(alBD, alBD, rrep)

                tc.strict_bb_all_engine_barrier()
                # ---- gates ----
                gps = pg.tile([2, 4 * H], F32, tag="gps")
                for cc in range(4):
                    cs = slice(512 * cc, 512 * cc + 512)
                    for ki in range(4):
                        nc.tensor.matmul(gps[:, cs],
                                         hcT[:, ki, :], decHT[:, ki, cs],
                                         start=(ki == 0), stop=False)
                    for ki, (r0, r1) in enumerate(ECH):
                        nc.tensor.matmul(gps[:, cs],
                                         mtT[: r1 - r0, ki, :],
                                         decMT[: r1 - r0, ki, cs],
                                         start=False, stop=False)
                    nc.tensor.matmul(gps[:, cs],
                                     alBD.bitcast(F32R), P_sb[:, cs],
                                     start=False, stop=False)
                    nc.tensor.matmul(gps[:, cs],
                                     identr[:TBL, 2 * t : 2 * t + 2],
                                     XD_sb[:, cs],
                                     start=False, stop=True)

                c_new = stp.tile([2, H], F32, tag="c_bm")
                h_new = stp.tile([2, H], F32, tag="h_bm")
                lstm_pointwise(gps, c_bm, c_new, h_new)
                c_bm, h_bm = c_new, h_new
                if not fused:
                    nc.sync.dma_start(hs[t, :, :], h_new)
                hcT = stp.tile([128, 8, 2], F32R, tag="hcT")
                tph = pt.tile([128, 16], F32, tag="tp")
                for k, (r0, r1) in enumerate(HCH):
                    nc.tensor.transpose(tph[:, 2 * k : 2 * k + 2],
                                        h_new[:, r0:r1], ident[:2, :2])
                    nc.tensor.transpose(tph[:, 8 + 2 * k : 8 + 2 * k + 2],
                                        c_new[:, r0:r1], ident[:2, :2])
                nc.vector.tensor_copy(hcT.bitcast(F32),
                                      tph.rearrange("p (k b) -> p k b", b=2))
                if fused:
                    nc.scalar.copy(
                        hsT_acc.rearrange("p k (b t) -> p k t b",
                                          t=steps)[:, :, t, :],
                        tph.rearrange("p (k b) -> p k b", b=2)[:, 0:4, :])

                tc.strict_bb_all_engine_barrier()
                # ---- mem blend: mem += gb * (cand - mem) ----
                for j, (r0, r1) in enumerate(ECH):
                    gb = pb.tile([128, MEMC], F32, tag="sm")
                    nc.tensor.matmul(gb[: r1 - r0, :], ones2[:, : r1 - r0],
                                     g_sb.bitcast(F32R), start=True, stop=True)
                    dd = rp.tile([128, MEMC], F32, tag="dd")
                    for b in range(2):
                        bc = slice(120 * b, 120 * b + 120)
                        nc.vector.tensor_scalar_add(
                            dd[: r1 - r0, bc],
                            candp[: r1 - r0, j, bc],
                            UT_sb[: r1 - r0, j, 2 * t + b : 2 * t + b + 1])
                    nc.vector.tensor_sub(dd[: r1 - r0, 0:240],
                                         dd[: r1 - r0, 0:240],
                                         memT[: r1 - r0, j, 0:240])
                    nc.vector.tensor_mul(dd[: r1 - r0, 0:240],
                                         dd[: r1 - r0, 0:240],
                                         gb[: r1 - r0, 0:240])
                    nc.vector.tensor_add(memT[: r1 - r0, j, 0:240],
                                         memT[: r1 - r0, j, 0:240],
                                         dd[: r1 - r0, 0:240])

            rctx.close()
            if fused:
                F16 = mybir.dt.float16
                LTOK = BL * steps          # local token cols (128)
                dpool = ctx.enter_context(
                    tc.tile_pool(name="dramp", bufs=1, space="DRAM"))
                hs_locT = dpool.tile([H, LTOK], F16, tag="hs_locT")
                hs_allT = dpool.tile([NC * H, LTOK], F16, tag="hs_allT")
                nc.gpsimd.dma_start(
                    hs_locT.rearrange("(k p) n -> p k n", p=128), hsT_acc)
                nc.gpsimd.collective_compute(
                    "AllGather", mybir.AluOpType.bypass,
                    replica_groups=[list(range(NC))],
                    ins=[hs_locT.opt()], outs=[hs_allT.opt()])

                vw = ctx.enter_context(tc.tile_pool(name="vw", bufs=1))
                vo = ctx.enter_context(tc.tile_pool(name="vo", bufs=3))
                vp = ctx.enter_context(
                    tc.tile_pool(name="vp", bufs=2, space="PSUM"))
                wT = vw.tile([128, 5, VS], F16, tag="wT")
                nc.sync.dma_start(
                    wT, d["woT"][0:640, :].rearrange("(c p) n -> p c n", p=128))
                hT = vw.tile([128, 5, B * L], F16, tag="hT")
                nc.vector.memset(hT[0:1, 4, :], 1.0)
                for c in range(NC):
                    nc.gpsimd.dma_start(
                        hT[:, 0:4, LTOK * c : LTOK * (c + 1)],
                        hs_allT[H * c : H * (c + 1), :].rearrange(
                            "(k p) n -> p k n", p=128))
                chunks = [(o, min(512, VS - o)) for o in range(0, VS, 512)]
                groups = [chunks[i : i + 4] for i in range(0, len(chunks), 4)]
                NBG = 2048
                for mb in range(B * L // 128):
                    for gi, grp in enumerate(groups):
                        g0 = grp[0][0]
                        gw = grp[-1][0] + grp[-1][1] - g0
                        ps = vp.tile([128, NBG], F32, tag="ps")
                        for k in range(5):
                            kw = 128 if k < 4 else 1
                            for (o, w_) in grp:
                                nc.tensor.matmul(
                                    ps[:, o - g0 : o - g0 + w_],
                                    hT[:kw, k, 128 * mb : 128 * mb + 128],
                                    wT[:kw, k, o : o + w_],
                                    start=(k == 0), stop=(k == 4))
                        amax = vo.tile([128, 1], F32, tag="amax")
                        nc.vector.tensor_reduce(
                            op=mybir.AluOpType.max, out=amax,
                            in_=ps[:, :gw], axis=mybir.AxisListType.X,
                            apply_absolute_value=True)
                        inv = vo.tile([128, 1], F32, tag="inv")
                        nc.vector.reciprocal(inv, amax)
                        fac = vo.tile([128, 1], F32, tag="fac")
                        nc.scalar.activation(fac, inv, AF.Copy, scale=127.0)
                        ot = vo.tile([128, NBG], mybir.dt.int8, tag="ot")
                        nc.scalar.activation(ot[:, :gw], ps[:, :gw], AF.Copy,
                                             scale=fac)
                        nc.sync.dma_start(
                            lg[128 * mb : 128 * mb + 128, g0 : g0 + gw],
                            ot[:, :gw])
                        nc.sync.dma_start(
                            lsc[128 * mb : 128 * mb + 128, gi : gi + 1], amax)
    return nc


def build_k2():
    F16 = mybir.dt.float16
    nc = bass.Bass(trn_type="TRN2", name="cteg_logits")
    hsT = nc.dram_tensor("hsT", [640, B * L], F32, kind="ExternalInput")
    woT = nc.dram_tensor("woT", [640, VS], F32, kind="ExternalInput")
    out = nc.dram_tensor("lg", [B * L, VS], F16, kind="ExternalOutput")
    NBG = 2048
    with tile.TileContext(nc) as tc:
        with ExitStack() as ctx:
            wpo = ctx.enter_context(tc.tile_pool(name="w", bufs=1))
            op = ctx.enter_context(tc.tile_pool(name="o", bufs=3))
            pp = ctx.enter_context(tc.tile_pool(name="p", bufs=2, space="PSUM"))
            hT = wpo.tile([128, 5, B * L], F32R, tag="hT")
            nc.sync.dma_start(
                hT, hsT[0:640, :].rearrange("(c p) n -> p c n", p=128).bitcast(F32R))
            wT = wpo.tile([128, 5, VS], F32R, tag="wT")
            nc.sync.dma_start(
                wT, woT[0:640, :].rearrange("(c p) n -> p c n", p=128).bitcast(F32R))
            chunks = [(o, min(512, VS - o)) for o in range(0, VS, 512)]
            groups = [chunks[i : i + 4] for i in range(0, len(chunks), 4)]
            for mb in range(B * L // 128):
                for grp in groups:
                    g0 = grp[0][0]
                    gw = grp[-1][0] + grp[-1][1] - g0
                    ps = pp.tile([128, NBG], F32, tag="ps")
                    for k in range(5):
                        kw = 128 if k < 4 else 1
                        for (o, w_) in grp:
                            nc.tensor.matmul(
                                ps[:, o - g0 : o - g0 + w_],
                                hT[:kw, k, 128 * mb : 128 * mb + 128],
                                wT[:kw, k, o : o + w_],
                                start=(k == 0), stop=(k == 4))
                    ot = op.tile([128, NBG], F16, tag="ot")
                    nc.scalar.copy(ot[:, :gw], ps[:, :gw])
                    nc.sync.dma_start(
                        out[128 * mb : 128 * mb + 128, g0 : g0 + gw],
                        ot[:, :gw])
    return nc


K1_WEIGHT_KEYS = (
    "enc_Wih_f", "enc_b_f", "enc_Wih_b", "enc_b_b", "enc_Whh_f", "enc_Whh_b",
    "dec_Wih", "dec_b", "dec_Whh", "Wp1", "bp1", "Wp2", "bp2", "Wep", "bep",
    "Wi1", "bi1", "Wmp", "bmp", "attn_v")


def _prep_shared(inputs):
    """Replicated k1 weight tensors (host layout/padding)."""
    f = lambda x: np.ascontiguousarray(np.asarray(x), dtype=np.float32)
    wih = f(inputs["dec_Wih"])
    shared = {
        "enc_xT_f": np.vstack([f(inputs["enc_Wih_f"]).T, f(inputs["enc_b_f"])[None]]),
        "enc_xT_b": np.vstack([f(inputs["enc_Wih_b"]).T, f(inputs["enc_b_b"])[None]]),
        "enc_hT_f": f(inputs["enc_Whh_f"]).T.copy(),
        "enc_hT_b": f(inputs["enc_Whh_b"]).T.copy(),
        "decXT": np.vstack([wih[:, :E].T, f(inputs["dec_b"])[None]]),
        "decAT": wih[:, E : E + H].T.copy(),
        "decMT": wih[:, E + H :].T.copy(),
        "decHT": f(inputs["dec_Whh"]).T.copy(),
        "wp1T_a": np.vstack([f(inputs["Wp1"]).T, f(inputs["bp1"])[None]]),
        "wp2T_a": np.vstack([f(inputs["Wp2"]).T, f(inputs["bp2"])[None]]),
        "wepT_a": np.vstack([f(inputs["Wep"]).T, f(inputs["bep"])[None]]),
        "wi1T_a": np.vstack([f(inputs["Wi1"]).T,
                             (f(inputs["bi1"]) + f(inputs["bmp"]))[None]]),
        "wmpT": f(inputs["Wmp"]).T.copy(),
        "attn_vT": f(inputs["attn_v"])[:, None].copy(),
    }
    mask_attn = np.zeros((2 * T, BL), np.float32)
    for t in range(T):
        for b in range(BL):
            mask_attn[2 * t + b, b] = 1.0
    shared["mask_attn"] = mask_attn
    mask_memT = np.zeros((BL, MEMC), np.float32)
    for b in range(BL):
        mask_memT[b, 120 * b : 120 * (b + 1)] = 1.0
    shared["mask_memT"] = mask_memT
    pad_to = {"enc_xT_f": 384, "enc_xT_b": 384, "decXT": 384, "decMT": 384,
              "wp1T_a": 640, "wp2T_a": 640, "wepT_a": 640, "wi1T_a": 384,
              "wmpT": 384}
    for k, rows in pad_to.items():
        v = shared[k]
        shared[k] = np.pad(v, ((0, rows - v.shape[0]), (0, 0)))
    return {k: np.ascontiguousarray(v, np.float32) for k, v in shared.items()}


def _prep_data(inputs):
    """Per-core embedding-gathered activations, concat over cores on axis 0."""
    emb = np.ascontiguousarray(np.asarray(inputs["embedding"]), np.float32)
    topic = np.asarray(inputs["topic"]).astype(np.int64)
    essay = np.asarray(inputs["essay_input"]).astype(np.int64)
    mems = np.asarray(inputs["mems"]).astype(np.int64)
    te = emb[topic]          # [B, T, E]
    ee = emb[essay]          # [B, L, E]
    me = emb[mems]           # [B, M, E]

    topicT = np.zeros((NC, 384, 2 * T), np.float32)
    essayT = np.zeros((NC, 384, 2 * L), np.float32)
    memT0 = np.zeros((NC, 384, MEMC), np.float32)
    topicT[:, E] = 1.0
    essayT[:, E] = 1.0
    # [B,S,E] -> per-core [E, 2*S] with (t,b) interleave on cols
    topicT[:, :E] = np.moveaxis(
        te.reshape(NC, BL, T, E), (1, 2, 3), (3, 2, 1)).reshape(NC, E, 2 * T)
    essayT[:, :E] = np.moveaxis(
        ee.reshape(NC, BL, L, E), (1, 2, 3), (3, 2, 1)).reshape(NC, E, 2 * L)
    memT0[:, :E, : 2 * M] = np.moveaxis(
        me.reshape(NC, BL, M, E), (1, 2, 3), (2, 3, 1)).reshape(NC, E, 2 * M)
    return {
        "topicT_a": np.ascontiguousarray(topicT).reshape(NC * 384, 2 * T),
        "essayT_a": np.ascontiguousarray(essayT).reshape(NC * 384, 2 * L),
        "memT0": np.ascontiguousarray(memT0).reshape(NC * 384, MEMC),
    }


def _prep_wout(inputs, dtype=np.float16):
    """Vocab-sharded transposed output projection, concat over cores."""
    wo = np.asarray(inputs["Wout"], np.float32)
    bo = np.asarray(inputs["bout"], np.float32)
    woT = np.zeros((NC, 640, VS), dtype)
    woT[:, :H] = wo.reshape(NC, VS, H).transpose(0, 2, 1)
    woT[:, H] = bo.reshape(NC, VS)
    return np.ascontiguousarray(woT).reshape(NC * 640, VS)


def _split_multi_waits(bir_json):
    """walrus in this env accepts at most ONE sync wait per instruction
    (S3_LW/CTRL_NO etc. reject more). Hoist extra waits onto same-engine
    NoOps inserted immediately before the instruction — sequencers execute
    in order, so the happens-before relation is preserved."""
    import json

    d = json.loads(bir_json)
    cnt = [0]
    for f in d["functions"]:
        for bb in f["blocks"]:
            out = []
            for inst in bb["instructions"]:
                si = inst.get("sync_info") or {}
                waits = si.get("on_wait") or []
                if len(waits) > 1 and inst["opcode"] != "ISA":
                    for w in waits[:-1]:
                        cnt[0] += 1
                        out.append({
                            "debug": inst.get("debug", 0),
                            "engine": inst["engine"],
                            "ins": [],
                            "outs": [],
                            "name": f"{inst['name']}-w{cnt[0]}",
                            "opcode": "NoOp",
                            "sync_info": {"on_update": [], "on_wait": [w]},
                        })
                    si["on_wait"] = [waits[-1]]
                    inst["sync_info"] = si
                out.append(inst)
            bb["instructions"] = out
    return json.dumps(d).encode()


def _patch_compile():
    import concourse.bass_utils as bu
    import concourse.bass2jax as b2j
    if getattr(bu, "_wait_patched", False):
        return
    orig = bu.compile_bir_kernel

    def patched(bir_json, tmpdir, neff_name="file.neff"):
        return orig(_split_multi_waits(bir_json), tmpdir, neff_name)

    bu.compile_bir_kernel = patched
    b2j.compile_bir_kernel = patched
    bu._wait_patched = True


# ---------------- persistent runner ----------------

import zlib


def _crc(*arrs):
    h = 0
    for a in arrs:
        a = np.ascontiguousarray(np.asarray(a))
        h = zlib.crc32(a.view(np.uint8).reshape(-1), h)
    return h


def _mesh():
    import jax
    from jax.sharding import Mesh
    if "mesh" not in _cache:
        devs = jax.devices()[:NC]
        assert len(devs) == NC
        _cache["mesh"] = Mesh(np.asarray(devs), ("core",))
    return _cache["mesh"]


def _meta(nc_obj):
    import jax
    partition_name = (nc_obj.partition_id_tensor.name
                      if nc_obj.partition_id_tensor else None)
    in_names, out_names, out_avals = [], [], []
    for alloc in nc_obj.m.functions[0].allocations:
        if not isinstance(alloc, mybir.MemoryLocationSet):
            continue
        name = alloc.memorylocations[0].name
        if alloc.kind == "ExternalInput":
            if name != partition_name:
                in_names.append(name)
        elif alloc.kind == "ExternalOutput":
            out_names.append(name)
            out_avals.append(jax.core.ShapedArray(
                tuple(alloc.tensor_shape), mybir.dt.np(alloc.dtype)))
    return in_names, out_names, out_avals, partition_name


def _make_fn(nc_obj, core_sharded_names):
    """Jitted SPMD launcher for a finalized bass module. Outputs are fully
    written by our kernels, so no donated zero buffers are passed."""
    import jax
    from jax.experimental.shard_map import shard_map
    from jax.sharding import PartitionSpec as P
    from concourse.bass2jax import _bass_exec_p, partition_id_tensor

    in_names, out_names, out_avals, partition_name = _meta(nc_obj)
    bind_names = tuple(in_names) + ((partition_name,) if partition_name else ())

    def _body(*args):
        operands = list(args)
        if partition_name:
            operands.append(partition_id_tensor())
        outs = _bass_exec_p.bind(
            *operands, out_avals=tuple(out_avals), in_names=bind_names,
            out_names=tuple(out_names), lowering_input_output_aliases=(),
            sim_require_finite=True, sim_require_nnan=True, nc=nc_obj)
        return tuple(outs)

    mesh = _mesh()
    in_specs = tuple(P("core") if n in core_sharded_names else P()
                     for n in in_names)
    out_specs = (P("core"),) * len(out_names)
    fn = jax.jit(shard_map(_body, mesh=mesh, in_specs=in_specs,
                           out_specs=out_specs, check_rep=False))
    return fn, in_names, out_names


def _upload_rep(np_map):
    """Host -> dev0 (1x over the wire) -> all-device replicate (D2D)."""
    import jax
    from jax.sharding import NamedSharding, PartitionSpec as P
    mesh = _mesh()
    vals = list(np_map.values())
    on0 = jax.device_put(vals, jax.devices()[0])
    jax.block_until_ready(on0)
    rep = jax.device_put(on0, NamedSharding(mesh, P()))
    jax.block_until_ready(rep)
    return dict(zip(np_map.keys(), rep))


def _upload_shard(np_list):
    import jax
    from jax.sharding import NamedSharding, PartitionSpec as P
    s = NamedSharding(_mesh(), P("core"))
    out = jax.device_put(np_list, s)
    jax.block_until_ready(out)
    return out


def _get_gather_fn():
    """hs [NC*L, BL, H] sharded-by-core -> hsT [640, B*L] replicated."""
    import jax
    import jax.numpy as jnp
    from functools import partial
    from jax.sharding import NamedSharding, PartitionSpec as P
    if "gather_fn" in _cache:
        return _cache["gather_fn"]
    s_rep = NamedSharding(_mesh(), P())

    @partial(jax.jit, out_shardings=s_rep)
    def g(hs):
        x = hs.reshape(NC, L, BL, H).transpose(0, 2, 1, 3).reshape(B * L, H)
        hT = x.T
        ones = jnp.ones((1, B * L), jnp.float32)
        pad = jnp.zeros((640 - H - 1, B * L), jnp.float32)
        return jnp.concatenate([hT, ones, pad], axis=0)

    _cache["gather_fn"] = g
    return g


def _upload_weights(inputs, wo_dtype=np.float16):
    k1key = _crc(*(inputs[k] for k in K1_WEIGHT_KEYS))
    if _cache.get("k1key") != k1key:
        _cache["k1w"] = _upload_rep(_prep_shared(inputs))
        _cache["k1key"] = k1key
    k2key = (_crc(inputs["Wout"], inputs["bout"]), np.dtype(wo_dtype).str)
    if _cache.get("k2key") != k2key:
        _cache["woT"] = _upload_shard([_prep_wout(inputs, wo_dtype)])[0]
        _cache["k2key"] = k2key


def _assemble(lg):
    lg_np = np.asarray(lg).reshape(NC, B * L, VS)       # float16
    out = np.empty((B * L, V), np.float32)
    for c in range(NC):
        out[:, VS * c : VS * (c + 1)] = lg_np[c]
    return out.reshape(B, L, V)


def _kernel_fused(inputs):
    if "kf_fn" not in _cache:
        _cache["kf"] = build_k1(fused=True)
        _cache["kf_fn"] = _make_fn(
            _cache["kf"], {"topicT_a", "essayT_a", "memT0", "woT"})
    _upload_weights(inputs, np.float16)
    data = _prep_data(inputs)
    ddev = dict(zip(data.keys(), _upload_shard(list(data.values()))))
    fn, innames, _ = _cache["kf_fn"]
    ops = [ddev[n] if n in ddev
           else (_cache["woT"] if n == "woT" else _cache["k1w"][n])
           for n in innames]
    lg, lsc = fn(*ops)
    lg_np = np.asarray(lg).reshape(NC, B * L, VS)       # int8
    sc_np = np.asarray(lsc).reshape(NC, B * L, 2).astype(np.float32) / 127.0
    out = np.empty((B * L, V), np.float32)
    for c in range(NC):
        for gi, (g0, g1) in enumerate(((0, 2048), (2048, VS))):
            np.multiply(lg_np[c][:, g0:g1], sc_np[c][:, gi : gi + 1],
                        out=out[:, VS * c + g0 : VS * c + g1])
    return out.reshape(B, L, V)


def _kernel_split(inputs):
    if "k1_fn" not in _cache:
        _cache["k1"] = build_k1()
        _cache["k1_fn"] = _make_fn(
            _cache["k1"], {"topicT_a", "essayT_a", "memT0"})
    if "k2_fn" not in _cache:
        _cache["k2"] = build_k2()
        _cache["k2_fn"] = _make_fn(_cache["k2"], {"woT"})
    _upload_weights(inputs, np.float32)
    data = _prep_data(inputs)
    ddev = dict(zip(data.keys(), _upload_shard(list(data.values()))))

    fn1, in1, _ = _cache["k1_fn"]
    ops1 = [ddev[n] if n in ddev else _cache["k1w"][n] for n in in1]
    (hs,) = fn1(*ops1)

    hsT = _get_gather_fn()(hs)

    fn2, in2, _ = _cache["k2_fn"]
    ops2 = [hsT if n == "hsT" else _cache["woT"] for n in in2]
    (lg,) = fn2(*ops2)
    return _assemble(lg)


def kernel(**inputs):
    _patch_compile()
    from concourse.bass2jax import install_neuronx_cc_hook
    install_neuronx_cc_hook()

    if not _cache.get("fused_broken"):
        try:
            return _kernel_fused(inputs)
        except Exception:
            import traceback
            traceback.print_exc()
            _cache["fused_broken"] = True
    return _kernel_split(inputs)



# revision 34
# speedup vs baseline: 1.8679x; 1.1605x over previous
"""CTEG kernel for 8x TRN2 NeuronCores.

K1 (SPMD, 8 cores): data-parallel recurrence (2 batch rows/core): encoder
   (bi-LSTM over T=8) + 64-step decoder with memory network + attention,
   emitting decoder hidden states hs [64, 2, 512].
K2 (SPMD, 8 cores): vocab-sharded projection: each core computes
   logits[:, :, c*4000:(c+1)*4000] = hs_all @ Wout_c.T + bout_c.

Host side: embedding gathers, weight transposes, shard assembly.
"""

import sys

sys.path.insert(0, "/opt/trn_rl_repo")

from contextlib import ExitStack

import numpy as np

import concourse.bass as bass
import concourse.mybir as mybir
import concourse.tile as tile
from concourse.masks import make_identity

B, T, L, V, E, H, A, M = 16, 8, 64, 32000, 300, 512, 128, 120
NC = 8
BL = B // NC          # 2 batch rows per core
VS = V // NC          # 4000 vocab rows per core
F32 = mybir.dt.float32
F32R = mybir.dt.float32  # fp32r needs rounded producers; plain fp32 for now
AF = mybir.ActivationFunctionType
MEMC = 256            # B*M=240 padded to 256 (fp32r needs free>=256 for 1cyc/row)
ECH = [(0, 128), (128, 256), (256, 300)]             # E row chunks
EACH = [(0, 128), (128, 256), (256, 301)]            # E+1 (bias row) chunks
HCH = [(0, 128), (128, 256), (256, 384), (384, 512)]

_cache = {}


def _chunked_load(nc, pool, dram, chunks, ncols, tag, dtype=F32R, cols=None):
    # dram is padded to len(chunks)*128 rows; single DMA, chunk-major layout
    nch = len(chunks)
    t_ = pool.tile([128, nch, ncols], dtype, tag=tag)
    c0, c1 = (0, ncols) if cols is None else cols
    src = dram[0 : 128 * nch, c0:c1].rearrange("(c p) n -> p c n", p=128)
    if dtype == F32R:
        src = src.bitcast(F32R)
    nc.sync.dma_start(t_, src)
    return t_


def build_k1(steps=L, tsteps=T, fused=False):
    nc = bass.Bass(trn_type="TRN2", name="cteg_fused" if fused else "cteg_rec",
                   num_devices=NC if fused else None)
    d = {}

    def inp(name, shape):
        d[name] = nc.dram_tensor(name, list(shape), F32, kind="ExternalInput")
        return d[name]

    TB = 2 * tsteps
    # packed per-core activations: cols [0:TB]=topicT, [TB:TB+2L]=essayT,
    # [TB+2L:TB+2L+MEMC]=memT0
    inp("actT", (384, TB + 2 * steps + MEMC))
    inp("enc_xT_f", (384, 4 * H))
    inp("enc_xT_b", (384, 4 * H))
    inp("enc_hT_f", (H, 4 * H))
    inp("enc_hT_b", (H, 4 * H))
    inp("decXT", (384, 4 * H))
    inp("decHT", (H, 4 * H))
    inp("decMT", (384, 4 * H))
    inp("decAT", (H, 4 * H))
    inp("wp1T_a", (640, E))
    inp("wp2T_a", (640, A))
    inp("wepT_a", (640, A))
    inp("wi1T_a", (384, E))
    inp("wmpT", (384, E))
    inp("attn_vT", (A, 1))
    inp("mask_attn", (TB, BL))      # [(t,b), b'] = (b==b')
    inp("mask_memT", (BL, MEMC))    # [b', c] = (c//120==b'), pad cols 0
    if fused:
        d["woT"] = nc.dram_tensor("woT", [640, VS], mybir.dt.float16,
                                  kind="ExternalInput")
        # int8 logits + per-(row, col-group) absmax scales; host dequantizes
        lg = nc.dram_tensor("lg", [B * L, VS], mybir.dt.int8,
                            kind="ExternalOutput")
        lsc = nc.dram_tensor("lsc", [B * L, 2], F32, kind="ExternalOutput")
    else:
        hs = nc.dram_tensor("hs", [steps, BL, H], F32, kind="ExternalOutput")

    with tile.TileContext(nc) as tc:
        with ExitStack() as ctx:
            wp = ctx.enter_context(tc.tile_pool(name="wts", bufs=1))
            sp = ctx.enter_context(tc.tile_pool(name="big", bufs=1))
            stp = ctx.enter_context(tc.tile_pool(name="state", bufs=3))
            rp = ctx.enter_context(tc.tile_pool(name="roll", bufs=4))
            sgp = ctx.enter_context(tc.tile_pool(name="sigp", bufs=2))
            # recurrence-phase pools (PSUM + decoder weights); closed before
            # the fused vocab-projection phase to free PSUM banks and SBUF
            rctx = ExitStack()
            pg = rctx.enter_context(tc.tile_pool(name="psg", bufs=1, space="PSUM"))
            pb = rctx.enter_context(tc.tile_pool(name="psb", bufs=1, space="PSUM"))
            pt = rctx.enter_context(tc.tile_pool(name="pst", bufs=1, space="PSUM"))

            # ---- small resident constants ----
            topicT = _chunked_load(nc, wp, d["actT"], EACH, TB, "topicT",
                                   cols=(0, TB))
            essayT = _chunked_load(nc, wp, d["actT"], EACH, 2 * steps, "essayT",
                                   cols=(TB, TB + 2 * steps))
            HACH = [(0, 128), (128, 256), (256, 384), (384, 512), (512, 513)]
            wp1T = _chunked_load(nc, wp, d["wp1T_a"], HACH, E, "wp1T")
            wp2T = _chunked_load(nc, wp, d["wp2T_a"], HACH, A, "wp2T")
            wepT = _chunked_load(nc, wp, d["wepT_a"], HACH, A, "wepT")
            wi1T = _chunked_load(nc, wp, d["wi1T_a"], EACH, E, "wi1T")
            wmpT = _chunked_load(nc, wp, d["wmpT"], ECH, E, "wmpT")
            attn_vT = wp.tile([A, 1], F32R, tag="attn_vT")
            nc.sync.dma_start(attn_vT, d["attn_vT"][:, :].bitcast(F32R))
            mask_attn = wp.tile([TB, BL], F32, tag="mask_attn")
            nc.sync.dma_start(mask_attn, d["mask_attn"][:, :])
            mask_memT = wp.tile([BL, MEMC], F32, tag="mask_memT")
            nc.sync.dma_start(mask_memT, d["mask_memT"][:, :])
            mask_memTr = mask_memT.bitcast(F32R)

            ident = wp.tile([128, 128], F32, tag="ident")
            make_identity(nc, ident)
            identr = ident.bitcast(F32R)
            ones2f = wp.tile([2, 128], F32, tag="ones2")
            nc.vector.memset(ones2f, 1.0)
            ones2 = ones2f.bitcast(F32R)

            memT = sp.tile([128, 3, MEMC], F32, tag="memT")
            nc.sync.dma_start(
                memT, d["actT"][0:384,
                                TB + 2 * steps : TB + 2 * steps + MEMC].rearrange(
                    "(c p) n -> p c n", p=128))
            memTr = memT.bitcast(F32R)

            h_bm = stp.tile([2, H], F32, tag="h_bm")
            c_bm = stp.tile([2, H], F32, tag="c_bm")
            # enc_outs stored transposed: eoT[:, k, 2t+b] = enc_outs[b, t, 128k+p]
            eoT = sp.tile([128, 4, TB], F32, tag="eoT")
            if fused:
                # hsT_acc[p, k, b*steps+t] = dec h_t[b, 128k+p] (f16 for the
                # fp16 vocab projection; recurrence itself stays fp32)
                hsT_acc = sp.tile([128, 4, BL * steps], mybir.dt.float16,
                                  tag="hsT_acc")

            def lstm_pointwise(gate_ps, cprev, cnext, hnext):
                # gate_ps [2, 4H] flat: i|f|g|o
                sig = sgp.tile([2, 4 * H], F32, tag="sig")
                nc.scalar.activation(sig[:, 0 : 2 * H], gate_ps[:, 0 : 2 * H],
                                     AF.Sigmoid)
                nc.scalar.activation(sig[:, 2 * H : 3 * H],
                                     gate_ps[:, 2 * H : 3 * H], AF.Tanh)
                nc.scalar.activation(sig[:, 3 * H : 4 * H],
                                     gate_ps[:, 3 * H : 4 * H], AF.Sigmoid)
                tmp = rp.tile([2, H], F32, tag="ctmp")
                nc.vector.tensor_mul(cnext, sig[:, H : 2 * H], cprev)
                nc.vector.tensor_mul(tmp, sig[:, 0:H], sig[:, 2 * H : 3 * H])
                nc.vector.tensor_add(cnext, cnext, tmp)
                tc2 = rp.tile([2, H], F32, tag="tc2")
                nc.scalar.activation(tc2, cnext, AF.Tanh)
                nc.vector.tensor_mul(hnext, sig[:, 3 * H : 4 * H], tc2)

            # ================= ENCODER =================
            hfin = {}
            cfin = {}
            with ExitStack() as ectx:
                eps2 = ectx.enter_context(tc.tile_pool(name="encs", bufs=4))
                for dr in ("f", "b"):
                    with ExitStack() as dctx:
                        epd = dctx.enter_context(
                            tc.tile_pool(name=f"encw{dr}", bufs=1))
                        xsb = epd.tile([TB, 4 * H], F32R, tag="xsb")
                        with ExitStack() as xctx:
                            xp = xctx.enter_context(
                                tc.tile_pool(name=f"encx{dr}", bufs=1))
                            xpp = xctx.enter_context(
                                tc.tile_pool(name=f"encxp{dr}", bufs=1,
                                             space="PSUM"))
                            ew = _chunked_load(nc, xp, d[f"enc_xT_{dr}"], EACH,
                                               4 * H, "ew")
                            for hf_ in range(2):
                                xps = xpp.tile([TB, 2 * H], F32, tag="xps")
                                for ki, (r0, r1) in enumerate(EACH):
                                    for c2 in range(2):
                                        cc = 2 * hf_ + c2
                                        nc.tensor.matmul(
                                            xps[:, 512 * c2 : 512 * c2 + 512],
                                            topicT[: r1 - r0, ki, :],
                                            ew[: r1 - r0, ki,
                                               512 * cc : 512 * cc + 512],
                                            start=(ki == 0), stop=(ki == 2))
                                nc.scalar.copy(
                                    xsb.bitcast(F32)[:, 1024 * hf_ :
                                                     1024 * hf_ + 1024], xps)
                        ehw = _chunked_load(
                            nc, epd, d[f"enc_hT_{dr}"],
                            [(128 * k, 128 * k + 128) for k in range(4)],
                            4 * H, "ehw")
                        hT0 = eps2.tile([128, 4, 2], F32R, tag="ehT")
                        nc.vector.memset(hT0.bitcast(F32), 0.0)
                        hT = None
                        cd = eps2.tile([2, H], F32, tag="ecd")
                        nc.vector.memset(cd, 0.0)
                        for s in range(tsteps):
                            t = s if dr == "f" else tsteps - 1 - s
                            tc.strict_bb_all_engine_barrier()
                            gps = pg.tile([2, 4 * H], F32, tag="gps")
                            if s == 0:
                                hT_prev = hT0
                            elif dr == "f":
                                hT_prev = eoT.bitcast(F32R)[
                                    :, :, 2 * (t - 1) : 2 * (t - 1) + 2]
                            else:
                                hT_prev = hT
                            for cc in range(4):
                                cs = slice(512 * cc, 512 * cc + 512)
                                for ki in range(4):
                                    nc.tensor.matmul(
                                        gps[:, cs],
                                        hT_prev[:, ki, :], ehw[:, ki, cs],
                                        start=(ki == 0), stop=False)
                                nc.tensor.matmul(
                                    gps[:, cs],
                                    identr[:TB, 2 * t : 2 * t + 2],
                                    xsb[:, cs],
                                    start=False, stop=True)
                            cnew = eps2.tile([2, H], F32, tag="ecn")
                            hnew = eps2.tile([2, H], F32, tag="ehn")
                            lstm_pointwise(gps, cd, cnew, hnew)
                            cd = cnew
                            tp = pt.tile([128, 8], F32, tag="tp")
                            for k, (r0, r1) in enumerate(HCH):
                                nc.tensor.transpose(
                                    tp[:, 2 * k : 2 * k + 2],
                                    hnew[:, r0:r1], ident[:2, :2])
                            tdst = eoT[:, :, 2 * t : 2 * t + 2]
                            tsrc = tp.rearrange("p (k b) -> p k b", b=2)
                            if dr == "f":
                                nc.vector.tensor_copy(tdst, tsrc)
                            else:
                                nc.vector.tensor_add(tdst, tdst, tsrc)
                            if s < tsteps - 1:
                                if dr == "f":
                                    hT = None  # fwd reads eoT directly
                                else:
                                    hT = eps2.tile([128, 4, 2], F32R, tag="ehT")
                                    nc.vector.tensor_copy(hT.bitcast(F32), tsrc)
                            else:
                                hfin[dr] = hnew
                        cfin[dr] = cd
                nc.vector.tensor_add(h_bm, hfin["f"], hfin["b"])
                nc.vector.tensor_add(c_bm, cfin["f"], cfin["b"])

            # dec weights in a pool opened after encoder pools closed
            H4CH = [(128 * k, 128 * k + 128) for k in range(4)]
            dwp = rctx.enter_context(tc.tile_pool(name="decw", bufs=1))
            decXT = _chunked_load(nc, dwp, d["decXT"], EACH, 4 * H, "decXT")
            decHT = _chunked_load(nc, dwp, d["decHT"], H4CH, 4 * H, "decHT")
            decMT = _chunked_load(nc, dwp, d["decMT"], ECH, 4 * H, "decMT")

            # hcT: chunks 0-3 = hT, 4-7 = cT
            hcT = stp.tile([128, 8, 2], F32R, tag="hcT")
            tp0 = pt.tile([128, 16], F32, tag="tp")
            for k, (r0, r1) in enumerate(HCH):
                nc.tensor.transpose(tp0[:, 2 * k : 2 * k + 2], h_bm[:, r0:r1],
                                    ident[:2, :2])
                nc.tensor.transpose(tp0[:, 8 + 2 * k : 8 + 2 * k + 2],
                                    c_bm[:, r0:r1], ident[:2, :2])
            nc.vector.tensor_copy(hcT.bitcast(F32),
                                  tp0.rearrange("p (k b) -> p k b", b=2))

            tc.strict_bb_all_engine_barrier()
            # ---- precompute phase ----
            TBL = 2 * steps
            P_sb = sp.tile([TB, 4 * H], F32R, tag="P_sb")
            epT_sb = sp.tile([A, TB], F32, tag="epT_sb")
            UT_sb = sp.tile([128, 3, TBL], F32, tag="UT_sb")
            XD_sb = sp.tile([TBL, 4 * H], F32R, tag="XD_sb")
            with ExitStack() as pctx:
                ppre = pctx.enter_context(
                    tc.tile_pool(name="pre", bufs=1, space="PSUM"))
                dap = pctx.enter_context(tc.tile_pool(name="decA", bufs=1))
                for hf_ in range(2):
                    decAT = dap.tile([128, 4, 1024], F32R, tag="decAT")
                    nc.sync.dma_start(
                        decAT,
                        d["decAT"][:, 1024 * hf_ : 1024 * hf_ + 1024].rearrange(
                            "(c p) n -> p c n", p=128).bitcast(F32R))
                    pps = ppre.tile([TB, 2 * H], F32, tag="pre")
                    for ki in range(4):
                        for c2 in range(2):
                            nc.tensor.matmul(
                                pps[:, 512 * c2 : 512 * c2 + 512], eoT.bitcast(F32R)[:, ki, :],
                                decAT[:, ki, 512 * c2 : 512 * c2 + 512],
                                start=(ki == 0), stop=(ki == 3))
                    nc.scalar.copy(
                        P_sb.bitcast(F32)[:, 1024 * hf_ : 1024 * hf_ + 1024], pps)

                # enc_procT [A, TB] (A-major): lhsT = wepT chunks, rhs = eoT (+ones)
                eph = ppre.tile([A, TB], F32, tag="pre")
                for ki in range(4):
                    nc.tensor.matmul(eph, wepT[:, ki, :], eoT.bitcast(F32R)[:, ki, :],
                                     start=(ki == 0), stop=False)
                nc.tensor.matmul(eph, wepT[0:1, 4, :], ones2[0:1, 0:TB],
                                 start=False, stop=True)
                nc.vector.tensor_copy(epT_sb, eph)

                for j, (c0, c1) in enumerate(ECH):
                    ups = ppre.tile([128, TBL], F32, tag="pre")
                    for ki, (r0, r1) in enumerate(EACH):
                        nc.tensor.matmul(ups[: c1 - c0, :],
                                         wi1T[: r1 - r0, ki, c0:c1],
                                         essayT[: r1 - r0, ki, :],
                                         start=(ki == 0), stop=(ki == 2))
                    nc.scalar.copy(UT_sb[: c1 - c0, j, :], ups[: c1 - c0, :])

                for cc in range(4):
                    xps2 = ppre.tile([TBL, H], F32, tag="pre")
                    for ki, (r0, r1) in enumerate(EACH):
                        nc.tensor.matmul(xps2, essayT[: r1 - r0, ki, :],
                                         decXT[: r1 - r0, ki,
                                               512 * cc : 512 * cc + 512],
                                         start=(ki == 0), stop=(ki == 2))
                    nc.scalar.copy(XD_sb.bitcast(F32)[:, 512 * cc : 512 * cc + 512],
                                   xps2)

            pc = rctx.enter_context(tc.tile_pool(name="psc", bufs=1, space="PSUM"))

            # ================= DECODER =================
            for t in range(steps):
                tc.strict_bb_all_engine_barrier()
                # ---- mem write pipeline (h-independent) ----
                candp = pc.tile([128, 3, MEMC], F32, tag="candp")
                for j, (c0, c1) in enumerate(ECH):
                    for ki, (r0, r1) in enumerate(ECH):
                        nc.tensor.matmul(candp[: c1 - c0, j, :],
                                         wmpT[: r1 - r0, ki, c0:c1],
                                         memTr[: r1 - r0, ki, :],
                                         start=(ki == 0), stop=(ki == 2))
                gps_m = pb.tile([2, MEMC], F32, tag="sm")
                for ki, (r0, r1) in enumerate(ECH):
                    nc.tensor.matmul(gps_m, essayT[: r1 - r0, ki, 2 * t : 2 * t + 2],
                                     memTr[: r1 - r0, ki, :],
                                     start=(ki == 0), stop=(ki == 2))
                g_sb = rp.tile([2, MEMC], F32, tag="g_sb")
                nc.scalar.activation(g_sb, gps_m, AF.Sigmoid)
                nc.vector.tensor_mul(g_sb, g_sb, mask_memT)

                tc.strict_bb_all_engine_barrier()
                # ---- mem read: v, sim, mt ----
                vps = pb.tile([2, E], F32, tag="sm")
                for ki in range(4):
                    nc.tensor.matmul(vps, hcT[:, ki, :], wp1T[:, ki, :],
                                     start=(ki == 0), stop=False)
                nc.tensor.matmul(vps, ones2[0:1, 0:2], wp1T[0:1, 4, :],
                                 start=False, stop=True)
                v_bm = rp.tile([2, E], F32, tag="v_bm")
                nc.scalar.activation(v_bm, vps, AF.Tanh)
                vT = rp.tile([128, 3, 2], F32R, tag="vT")
                tpv = pt.tile([128, 6], F32, tag="tp")
                for j, (r0, r1) in enumerate(ECH):
                    nc.tensor.transpose(tpv[: r1 - r0, 2 * j : 2 * j + 2],
                                        v_bm[:, r0:r1], ident[:2, :2])
                for j, (r0, r1) in enumerate(ECH):
                    nc.vector.tensor_copy(vT.bitcast(F32)[: r1 - r0, j, :],
                                          tpv[: r1 - r0, 2 * j : 2 * j + 2])
                sps = pb.tile([2, MEMC], F32, tag="sm")
                for ki, (r0, r1) in enumerate(ECH):
                    nc.tensor.matmul(sps, vT[: r1 - r0, ki, :],
                                     memTr[: r1 - r0, ki, :],
                                     start=(ki == 0), stop=(ki == 2))
                es = rp.tile([2, MEMC], F32, tag="es")
                nc.scalar.activation(es, sps, AF.Exp)
                den = rp.tile([2, 1], F32, tag="den")
                nc.vector.tensor_mul(es, es, mask_memT)
                nc.vector.tensor_reduce(op=mybir.AluOpType.add, out=den,
                                        in_=es, axis=mybir.AxisListType.X)
                nc.vector.reciprocal(den, den)
                nc.vector.tensor_scalar_mul(es, es, den)
                esr = es.bitcast(F32R)

                tc.strict_bb_all_engine_barrier()
                mtT = rp.tile([128, 3, 2], F32R, tag="mtT")
                junk = rp.tile([128, 120], F32, tag="junk")
                for j, (r0, r1) in enumerate(ECH):
                    arep = pb.tile([128, MEMC], F32, tag="sm")
                    nc.tensor.matmul(arep[: r1 - r0, :], ones2[:, : r1 - r0], esr,
                                     start=True, stop=True)
                    for b in range(2):
                        nc.vector.tensor_mul(
                            junk[: r1 - r0, :],
                            memT[: r1 - r0, j, 120 * b : 120 * b + 120],
                            arep[: r1 - r0, 120 * b : 120 * b + 120])
                        nc.vector.tensor_reduce(
                            op=mybir.AluOpType.add,
                            out=mtT.bitcast(F32)[: r1 - r0, j, b : b + 1],
                            in_=junk[: r1 - r0, :], axis=mybir.AxisListType.X)

                tc.strict_bb_all_engine_barrier()
                # ---- attention ----
                qps = pb.tile([A, 2], F32, tag="sm")
                for ki in range(4):
                    nc.tensor.matmul(qps, wp2T[:, ki, :], hcT[:, 4 + ki, :],
                                     start=(ki == 0), stop=False)
                nc.tensor.matmul(qps, wp2T[0:1, 4, :], ones2[0:1, 0:2],
                                 start=False, stop=True)
                qsb = rp.tile([A, 2], F32, tag="qsb")
                nc.vector.tensor_copy(qsb, qps)
                tha = rp.tile([A, TB], F32, tag="tha")
                for b in range(2):
                    nc.scalar.activation(
                        tha.rearrange("a (t b) -> a t b", b=2)[:, :, b],
                        epT_sb.rearrange("a (t b) -> a t b", b=2)[:, :, b],
                        AF.Tanh, bias=qsb[:, b : b + 1], scale=1.0)
                scps = pb.tile([1, TB], F32, tag="sm")
                nc.tensor.matmul(scps, attn_vT, tha.bitcast(F32R),
                                 start=True, stop=True)
                esc = rp.tile([1, TB], F32, tag="esc")
                nc.scalar.activation(esc, scps, AF.Exp)
                escT = pt.tile([TB, 1], F32, tag="tp")
                nc.tensor.transpose(escT, esc, ident[0:1, 0:1])
                escTs = rp.tile([TB, 1], F32, tag="escTs")
                nc.vector.tensor_copy(escTs, escT)
                sms = pb.tile([2, 1], F32, tag="sm")
                nc.tensor.matmul(sms, mask_attn.bitcast(F32R),
                                 escTs.bitcast(F32R), start=True, stop=True)
                rden = rp.tile([2, 1], F32, tag="rden")
                nc.vector.reciprocal(rden, sms)
                rrT = pt.tile([1, 2], F32, tag="tp")
                nc.tensor.transpose(rrT, rden, ident[:2, :2])
                rr_sb = rp.tile([1, 2], F32, tag="rr_sb")
                nc.vector.tensor_copy(rr_sb, rrT)
                rrep = pb.tile([TB, 2], F32, tag="sm")
                nc.tensor.matmul(rrep, ones2[0:1, 0:TB], rr_sb.bitcast(F32R),
                                 start=True, stop=True)
                alBD = rp.tile([TB, BL], F32, tag="alBD")
                nc.vector.tensor_scalar_mul(alBD, mask_attn, escTs)
                nc.vector.tensor_mul(alBD, alBD, rrep)

                tc.strict_bb_all_engine_barrier()
                # ---- gates ----
                gps = pg.tile([2, 4 * H], F32, tag="gps")
                for cc in range(4):
                    cs = slice(512 * cc, 512 * cc + 512)
                    for ki in range(4):
                        nc.tensor.matmul(gps[:, cs],
                                         hcT[:, ki, :], decHT[:, ki, cs],
                                         start=(ki == 0), stop=False)
                    for ki, (r0, r1) in enumerate(ECH):
                        nc.tensor.matmul(gps[:, cs],
                                         mtT[: r1 - r0, ki, :],
                                         decMT[: r1 - r0, ki, cs],
                                         start=False, stop=False)
                    nc.tensor.matmul(gps[:, cs],
                                     alBD.bitcast(F32R), P_sb[:, cs],
                                     start=False, stop=False)
                    nc.tensor.matmul(gps[:, cs],
                                     identr[:TBL, 2 * t : 2 * t + 2],
                                     XD_sb[:, cs],
                                     start=False, stop=True)

                c_new = stp.tile([2, H], F32, tag="c_bm")
                h_new = stp.tile([2, H], F32, tag="h_bm")
                lstm_pointwise(gps, c_bm, c_new, h_new)
                c_bm, h_bm = c_new, h_new
                if not fused:
                    nc.sync.dma_start(hs[t, :, :], h_new)
                hcT = stp.tile([128, 8, 2], F32R, tag="hcT")
                tph = pt.tile([128, 16], F32, tag="tp")
                for k, (r0, r1) in enumerate(HCH):
                    nc.tensor.transpose(tph[:, 2 * k : 2 * k + 2],
                                        h_new[:, r0:r1], ident[:2, :2])
                    nc.tensor.transpose(tph[:, 8 + 2 * k : 8 + 2 * k + 2],
                                        c_new[:, r0:r1], ident[:2, :2])
                nc.vector.tensor_copy(hcT.bitcast(F32),
                                      tph.rearrange("p (k b) -> p k b", b=2))
                if fused:
                    nc.scalar.copy(
                        hsT_acc.rearrange("p k (b t) -> p k t b",
                                          t=steps)[:, :, t, :],
                        tph.rearrange("p (k b) -> p k b", b=2)[:, 0:4, :])

                tc.strict_bb_all_engine_barrier()
                # ---- mem blend: mem += gb * (cand - mem) ----
                for j, (r0, r1) in enumerate(ECH):
                    gb = pb.tile([128, MEMC], F32, tag="sm")
                    nc.tensor.matmul(gb[: r1 - r0, :], ones2[:, : r1 - r0],
                                     g_sb.bitcast(F32R), start=True, stop=True)
                    dd = rp.tile([128, MEMC], F32, tag="dd")
                    for b in range(2):
                        bc = slice(120 * b, 120 * b + 120)
                        nc.vector.tensor_scalar_add(
                            dd[: r1 - r0, bc],
                            candp[: r1 - r0, j, bc],
                            UT_sb[: r1 - r0, j, 2 * t + b : 2 * t + b + 1])
                    nc.vector.tensor_sub(dd[: r1 - r0, 0:240],
                                         dd[: r1 - r0, 0:240],
                                         memT[: r1 - r0, j, 0:240])
                    nc.vector.tensor_mul(dd[: r1 - r0, 0:240],
                                         dd[: r1 - r0, 0:240],
                                         gb[: r1 - r0, 0:240])
                    nc.vector.tensor_add(memT[: r1 - r0, j, 0:240],
                                         memT[: r1 - r0, j, 0:240],
                                         dd[: r1 - r0, 0:240])

            rctx.close()
            if fused:
                F16 = mybir.dt.float16
                LTOK = BL * steps          # local token cols (128)
                dpool = ctx.enter_context(
                    tc.tile_pool(name="dramp", bufs=1, space="DRAM"))
                hs_locT = dpool.tile([H, LTOK], F16, tag="hs_locT")
                hs_allT = dpool.tile([NC * H, LTOK], F16, tag="hs_allT")
                nc.gpsimd.dma_start(
                    hs_locT.rearrange("(k p) n -> p k n", p=128), hsT_acc)
                nc.gpsimd.collective_compute(
                    "AllGather", mybir.AluOpType.bypass,
                    replica_groups=[list(range(NC))],
                    ins=[hs_locT.opt()], outs=[hs_allT.opt()])

                vw = ctx.enter_context(tc.tile_pool(name="vw", bufs=1))
                vo = ctx.enter_context(tc.tile_pool(name="vo", bufs=3))
                vp = ctx.enter_context(
                    tc.tile_pool(name="vp", bufs=2, space="PSUM"))
                wT = vw.tile([128, 5, VS], F16, tag="wT")
                nc.sync.dma_start(
                    wT, d["woT"][0:640, :].rearrange("(c p) n -> p c n", p=128))
                hT = vw.tile([128, 5, B * L], F16, tag="hT")
                nc.vector.memset(hT[0:1, 4, :], 1.0)
                for c in range(NC):
                    nc.gpsimd.dma_start(
                        hT[:, 0:4, LTOK * c : LTOK * (c + 1)],
                        hs_allT[H * c : H * (c + 1), :].rearrange(
                            "(k p) n -> p k n", p=128))
                chunks = [(o, min(512, VS - o)) for o in range(0, VS, 512)]
                groups = [chunks[i : i + 4] for i in range(0, len(chunks), 4)]
                NBG = 2048
                for mb in range(B * L // 128):
                    for gi, grp in enumerate(groups):
                        g0 = grp[0][0]
                        gw = grp[-1][0] + grp[-1][1] - g0
                        ps = vp.tile([128, NBG], F32, tag="ps")
                        for k in range(5):
                            kw = 128 if k < 4 else 1
                            for (o, w_) in grp:
                                nc.tensor.matmul(
                                    ps[:, o - g0 : o - g0 + w_],
                                    hT[:kw, k, 128 * mb : 128 * mb + 128],
                                    wT[:kw, k, o : o + w_],
                                    start=(k == 0), stop=(k == 4))
                        amax = vo.tile([128, 1], F32, tag="amax")
                        nc.vector.tensor_reduce(
                            op=mybir.AluOpType.max, out=amax,
                            in_=ps[:, :gw], axis=mybir.AxisListType.X,
                            apply_absolute_value=True)
                        inv = vo.tile([128, 1], F32, tag="inv")
                        nc.vector.reciprocal(inv, amax)
                        fac = vo.tile([128, 1], F32, tag="fac")
                        nc.scalar.activation(fac, inv, AF.Copy, scale=127.0)
                        ot = vo.tile([128, NBG], mybir.dt.int8, tag="ot")
                        nc.scalar.activation(ot[:, :gw], ps[:, :gw], AF.Copy,
                                             scale=fac)
                        nc.sync.dma_start(
                            lg[128 * mb : 128 * mb + 128, g0 : g0 + gw],
                            ot[:, :gw])
                        nc.sync.dma_start(
                            lsc[128 * mb : 128 * mb + 128, gi : gi + 1], amax)
    return nc


def build_k2():
    F16 = mybir.dt.float16
    nc = bass.Bass(trn_type="TRN2", name="cteg_logits")
    hsT = nc.dram_tensor("hsT", [640, B * L], F32, kind="ExternalInput")
    woT = nc.dram_tensor("woT", [640, VS], F32, kind="ExternalInput")
    out = nc.dram_tensor("lg", [B * L, VS], F16, kind="ExternalOutput")
    NBG = 2048
    with tile.TileContext(nc) as tc:
        with ExitStack() as ctx:
            wpo = ctx.enter_context(tc.tile_pool(name="w", bufs=1))
            op = ctx.enter_context(tc.tile_pool(name="o", bufs=3))
            pp = ctx.enter_context(tc.tile_pool(name="p", bufs=2, space="PSUM"))
            hT = wpo.tile([128, 5, B * L], F32R, tag="hT")
            nc.sync.dma_start(
                hT, hsT[0:640, :].rearrange("(c p) n -> p c n", p=128).bitcast(F32R))
            wT = wpo.tile([128, 5, VS], F32R, tag="wT")
            nc.sync.dma_start(
                wT, woT[0:640, :].rearrange("(c p) n -> p c n", p=128).bitcast(F32R))
            chunks = [(o, min(512, VS - o)) for o in range(0, VS, 512)]
            groups = [chunks[i : i + 4] for i in range(0, len(chunks), 4)]
            for mb in range(B * L // 128):
                for grp in groups:
                    g0 = grp[0][0]
                    gw = grp[-1][0] + grp[-1][1] - g0
                    ps = pp.tile([128, NBG], F32, tag="ps")
                    for k in range(5):
                        kw = 128 if k < 4 else 1
                        for (o, w_) in grp:
                            nc.tensor.matmul(
                                ps[:, o - g0 : o - g0 + w_],
                                hT[:kw, k, 128 * mb : 128 * mb + 128],
                                wT[:kw, k, o : o + w_],
                                start=(k == 0), stop=(k == 4))
                    ot = op.tile([128, NBG], F16, tag="ot")
                    nc.scalar.copy(ot[:, :gw], ps[:, :gw])
                    nc.sync.dma_start(
                        out[128 * mb : 128 * mb + 128, g0 : g0 + gw],
                        ot[:, :gw])
    return nc


K1_WEIGHT_KEYS = (
    "enc_Wih_f", "enc_b_f", "enc_Wih_b", "enc_b_b", "enc_Whh_f", "enc_Whh_b",
    "dec_Wih", "dec_b", "dec_Whh", "Wp1", "bp1", "Wp2", "bp2", "Wep", "bep",
    "Wi1", "bi1", "Wmp", "bmp", "attn_v")


def _prep_shared(inputs):
    """Replicated k1 weight tensors (host layout/padding)."""
    f = lambda x: np.ascontiguousarray(np.asarray(x), dtype=np.float32)
    wih = f(inputs["dec_Wih"])
    shared = {
        "enc_xT_f": np.vstack([f(inputs["enc_Wih_f"]).T, f(inputs["enc_b_f"])[None]]),
        "enc_xT_b": np.vstack([f(inputs["enc_Wih_b"]).T, f(inputs["enc_b_b"])[None]]),
        "enc_hT_f": f(inputs["enc_Whh_f"]).T.copy(),
        "enc_hT_b": f(inputs["enc_Whh_b"]).T.copy(),
        "decXT": np.vstack([wih[:, :E].T, f(inputs["dec_b"])[None]]),
        "decAT": wih[:, E : E + H].T.copy(),
        "decMT": wih[:, E + H :].T.copy(),
        "decHT": f(inputs["dec_Whh"]).T.copy(),
        "wp1T_a": np.vstack([f(inputs["Wp1"]).T, f(inputs["bp1"])[None]]),
        "wp2T_a": np.vstack([f(inputs["Wp2"]).T, f(inputs["bp2"])[None]]),
        "wepT_a": np.vstack([f(inputs["Wep"]).T, f(inputs["bep"])[None]]),
        "wi1T_a": np.vstack([f(inputs["Wi1"]).T,
                             (f(inputs["bi1"]) + f(inputs["bmp"]))[None]]),
        "wmpT": f(inputs["Wmp"]).T.copy(),
        "attn_vT": f(inputs["attn_v"])[:, None].copy(),
    }
    mask_attn = np.zeros((2 * T, BL), np.float32)
    for t in range(T):
        for b in range(BL):
            mask_attn[2 * t + b, b] = 1.0
    shared["mask_attn"] = mask_attn
    mask_memT = np.zeros((BL, MEMC), np.float32)
    for b in range(BL):
        mask_memT[b, 120 * b : 120 * (b + 1)] = 1.0
    shared["mask_memT"] = mask_memT
    pad_to = {"enc_xT_f": 384, "enc_xT_b": 384, "decXT": 384, "decMT": 384,
              "wp1T_a": 640, "wp2T_a": 640, "wepT_a": 640, "wi1T_a": 384,
              "wmpT": 384}
    for k, rows in pad_to.items():
        v = shared[k]
        shared[k] = np.pad(v, ((0, rows - v.shape[0]), (0, 0)))
    return {k: np.ascontiguousarray(v, np.float32) for k, v in shared.items()}


def _prep_data(inputs):
    """Per-core embedding-gathered activations, packed [NC*384, TB+2L+MEMC]."""
    emb = np.ascontiguousarray(np.asarray(inputs["embedding"]), np.float32)
    topic = np.asarray(inputs["topic"]).astype(np.int64)
    essay = np.asarray(inputs["essay_input"]).astype(np.int64)
    mems = np.asarray(inputs["mems"]).astype(np.int64)
    te = emb[topic]          # [B, T, E]
    ee = emb[essay]          # [B, L, E]
    me = emb[mems]           # [B, M, E]

    TB = 2 * T
    act = np.zeros((NC, 384, TB + 2 * L + MEMC), np.float32)
    act[:, E, 0 : TB + 2 * L] = 1.0
    # [B,S,E] -> per-core [E, 2*S] with (t,b) interleave on cols
    act[:, :E, 0:TB] = np.moveaxis(
        te.reshape(NC, BL, T, E), (1, 2, 3), (3, 2, 1)).reshape(NC, E, 2 * T)
    act[:, :E, TB : TB + 2 * L] = np.moveaxis(
        ee.reshape(NC, BL, L, E), (1, 2, 3), (3, 2, 1)).reshape(NC, E, 2 * L)
    act[:, :E, TB + 2 * L : TB + 2 * L + 2 * M] = np.moveaxis(
        me.reshape(NC, BL, M, E), (1, 2, 3), (2, 3, 1)).reshape(NC, E, 2 * M)
    return {"actT": act.reshape(NC * 384, TB + 2 * L + MEMC)}


def _prep_wout(inputs, dtype=np.float16):
    """Vocab-sharded transposed output projection, concat over cores."""
    wo = np.asarray(inputs["Wout"], np.float32)
    bo = np.asarray(inputs["bout"], np.float32)
    woT = np.zeros((NC, 640, VS), dtype)
    woT[:, :H] = wo.reshape(NC, VS, H).transpose(0, 2, 1)
    woT[:, H] = bo.reshape(NC, VS)
    return np.ascontiguousarray(woT).reshape(NC * 640, VS)


def _split_multi_waits(bir_json):
    """walrus in this env accepts at most ONE sync wait per instruction
    (S3_LW/CTRL_NO etc. reject more). Hoist extra waits onto same-engine
    NoOps inserted immediately before the instruction — sequencers execute
    in order, so the happens-before relation is preserved."""
    import json

    d = json.loads(bir_json)
    cnt = [0]
    for f in d["functions"]:
        for bb in f["blocks"]:
            out = []
            for inst in bb["instructions"]:
                si = inst.get("sync_info") or {}
                waits = si.get("on_wait") or []
                if len(waits) > 1 and inst["opcode"] != "ISA":
                    for w in waits[:-1]:
                        cnt[0] += 1
                        out.append({
                            "debug": inst.get("debug", 0),
                            "engine": inst["engine"],
                            "ins": [],
                            "outs": [],
                            "name": f"{inst['name']}-w{cnt[0]}",
                            "opcode": "NoOp",
                            "sync_info": {"on_update": [], "on_wait": [w]},
                        })
                    si["on_wait"] = [waits[-1]]
                    inst["sync_info"] = si
                out.append(inst)
            bb["instructions"] = out
    return json.dumps(d).encode()


def _patch_compile():
    import concourse.bass_utils as bu
    import concourse.bass2jax as b2j
    if getattr(bu, "_wait_patched", False):
        return
    orig = bu.compile_bir_kernel

    def patched(bir_json, tmpdir, neff_name="file.neff"):
        return orig(_split_multi_waits(bir_json), tmpdir, neff_name)

    bu.compile_bir_kernel = patched
    b2j.compile_bir_kernel = patched
    bu._wait_patched = True


# ---------------- persistent runner ----------------

import zlib


def _crc(*arrs):
    h = 0
    for a in arrs:
        a = np.ascontiguousarray(np.asarray(a))
        h = zlib.crc32(a.view(np.uint8).reshape(-1), h)
    return h


def _mesh():
    import jax
    from jax.sharding import Mesh
    if "mesh" not in _cache:
        devs = jax.devices()[:NC]
        assert len(devs) == NC
        _cache["mesh"] = Mesh(np.asarray(devs), ("core",))
    return _cache["mesh"]


def _meta(nc_obj):
    import jax
    partition_name = (nc_obj.partition_id_tensor.name
                      if nc_obj.partition_id_tensor else None)
    in_names, out_names, out_avals = [], [], []
    for alloc in nc_obj.m.functions[0].allocations:
        if not isinstance(alloc, mybir.MemoryLocationSet):
            continue
        name = alloc.memorylocations[0].name
        if alloc.kind == "ExternalInput":
            if name != partition_name:
                in_names.append(name)
        elif alloc.kind == "ExternalOutput":
            out_names.append(name)
            out_avals.append(jax.core.ShapedArray(
                tuple(alloc.tensor_shape), mybir.dt.np(alloc.dtype)))
    return in_names, out_names, out_avals, partition_name


def _make_fn(nc_obj, core_sharded_names):
    """Jitted SPMD launcher for a finalized bass module. Outputs are fully
    written by our kernels, so no donated zero buffers are passed."""
    import jax
    from jax.experimental.shard_map import shard_map
    from jax.sharding import PartitionSpec as P
    from concourse.bass2jax import _bass_exec_p, partition_id_tensor

    in_names, out_names, out_avals, partition_name = _meta(nc_obj)
    bind_names = tuple(in_names) + ((partition_name,) if partition_name else ())

    def _body(*args):
        operands = list(args)
        if partition_name:
            operands.append(partition_id_tensor())
        outs = _bass_exec_p.bind(
            *operands, out_avals=tuple(out_avals), in_names=bind_names,
            out_names=tuple(out_names), lowering_input_output_aliases=(),
            sim_require_finite=True, sim_require_nnan=True, nc=nc_obj)
        return tuple(outs)

    mesh = _mesh()
    in_specs = tuple(P("core") if n in core_sharded_names else P()
                     for n in in_names)
    out_specs = (P("core"),) * len(out_names)
    fn = jax.jit(shard_map(_body, mesh=mesh, in_specs=in_specs,
                           out_specs=out_specs, check_rep=False))
    return fn, in_names, out_names


def _upload_rep(np_map):
    """Host -> dev0 (1x over the wire) -> all-device replicate (D2D)."""
    import jax
    from jax.sharding import NamedSharding, PartitionSpec as P
    mesh = _mesh()
    vals = list(np_map.values())
    on0 = jax.device_put(vals, jax.devices()[0])
    jax.block_until_ready(on0)
    rep = jax.device_put(on0, NamedSharding(mesh, P()))
    jax.block_until_ready(rep)
    return dict(zip(np_map.keys(), rep))


def _upload_shard(np_list, block=True):
    import jax
    from jax.sharding import NamedSharding, PartitionSpec as P
    s = NamedSharding(_mesh(), P("core"))
    out = jax.device_put(np_list, s)
    if block:
        jax.block_until_ready(out)
    return out


def _get_gather_fn():
    """hs [NC*L, BL, H] sharded-by-core -> hsT [640, B*L] replicated."""
    import jax
    import jax.numpy as jnp
    from functools import partial
    from jax.sharding import NamedSharding, PartitionSpec as P
    if "gather_fn" in _cache:
        return _cache["gather_fn"]
    s_rep = NamedSharding(_mesh(), P())

    @partial(jax.jit, out_shardings=s_rep)
    def g(hs):
        x = hs.reshape(NC, L, BL, H).transpose(0, 2, 1, 3).reshape(B * L, H)
        hT = x.T
        ones = jnp.ones((1, B * L), jnp.float32)
        pad = jnp.zeros((640 - H - 1, B * L), jnp.float32)
        return jnp.concatenate([hT, ones, pad], axis=0)

    _cache["gather_fn"] = g
    return g


def _upload_weights(inputs, wo_dtype=np.float16):
    k1key = _crc(*(inputs[k] for k in K1_WEIGHT_KEYS))
    if _cache.get("k1key") != k1key:
        _cache["k1w"] = _upload_rep(_prep_shared(inputs))
        _cache["k1key"] = k1key
    k2key = (_crc(inputs["Wout"], inputs["bout"]), np.dtype(wo_dtype).str)
    if _cache.get("k2key") != k2key:
        _cache["woT"] = _upload_shard([_prep_wout(inputs, wo_dtype)])[0]
        _cache["k2key"] = k2key


def _assemble(lg):
    lg_np = np.asarray(lg).reshape(NC, B * L, VS)       # float16
    out = np.empty((B * L, V), np.float32)
    for c in range(NC):
        out[:, VS * c : VS * (c + 1)] = lg_np[c]
    return out.reshape(B, L, V)


def _kernel_fused(inputs):
    from concurrent.futures import ThreadPoolExecutor
    if "kf_fn" not in _cache:
        _cache["kf"] = build_k1(fused=True)
        _cache["kf_fn"] = _make_fn(_cache["kf"], {"actT", "woT"})
    if "pool" not in _cache:
        _cache["pool"] = ThreadPoolExecutor(2)
    data = _prep_data(inputs)
    (act_dev,) = _upload_shard([data["actT"]], block=False)  # overlaps crc
    _upload_weights(inputs, np.float16)
    fn, innames, _ = _cache["kf_fn"]
    ops = [act_dev if n == "actT"
           else (_cache["woT"] if n == "woT" else _cache["k1w"][n])
           for n in innames]
    lg, lsc = fn(*ops)
    sc_fut = _cache["pool"].submit(np.asarray, lsc)
    lg_np = np.asarray(lg).reshape(NC, B * L, VS)       # int8
    sc_np = np.asarray(sc_fut.result()).reshape(NC, B * L, 2)
    sc_np = sc_np.astype(np.float32) / 127.0
    out = np.empty((B * L, V), np.float32)
    for c in range(NC):
        for gi, (g0, g1) in enumerate(((0, 2048), (2048, VS))):
            np.multiply(lg_np[c][:, g0:g1], sc_np[c][:, gi : gi + 1],
                        out=out[:, VS * c + g0 : VS * c + g1])
    return out.reshape(B, L, V)


def _kernel_split(inputs):
    if "k1_fn" not in _cache:
        _cache["k1"] = build_k1()
        _cache["k1_fn"] = _make_fn(_cache["k1"], {"actT"})
    if "k2_fn" not in _cache:
        _cache["k2"] = build_k2()
        _cache["k2_fn"] = _make_fn(_cache["k2"], {"woT"})
    _upload_weights(inputs, np.float32)
    data = _prep_data(inputs)
    ddev = dict(zip(data.keys(), _upload_shard(list(data.values()))))

    fn1, in1, _ = _cache["k1_fn"]
    ops1 = [ddev[n] if n in ddev else _cache["k1w"][n] for n in in1]
    (hs,) = fn1(*ops1)

    hsT = _get_gather_fn()(hs)

    fn2, in2, _ = _cache["k2_fn"]
    ops2 = [hsT if n == "hsT" else _cache["woT"] for n in in2]
    (lg,) = fn2(*ops2)
    return _assemble(lg)


def kernel(**inputs):
    _patch_compile()
    from concourse.bass2jax import install_neuronx_cc_hook
    install_neuronx_cc_hook()

    if not _cache.get("fused_broken"):
        try:
            return _kernel_fused(inputs)
        except Exception:
            import traceback
            traceback.print_exc()
            _cache["fused_broken"] = True
    return _kernel_split(inputs)



# revision 36
# speedup vs baseline: 2.1216x; 1.1358x over previous
"""CTEG kernel for 8x TRN2 NeuronCores.

K1 (SPMD, 8 cores): data-parallel recurrence (2 batch rows/core): encoder
   (bi-LSTM over T=8) + 64-step decoder with memory network + attention,
   emitting decoder hidden states hs [64, 2, 512].
K2 (SPMD, 8 cores): vocab-sharded projection: each core computes
   logits[:, :, c*4000:(c+1)*4000] = hs_all @ Wout_c.T + bout_c.

Host side: embedding gathers, weight transposes, shard assembly.
"""

import sys

sys.path.insert(0, "/opt/trn_rl_repo")

from contextlib import ExitStack

import numpy as np

import concourse.bass as bass
import concourse.mybir as mybir
import concourse.tile as tile
from concourse.masks import make_identity

B, T, L, V, E, H, A, M = 16, 8, 64, 32000, 300, 512, 128, 120
NC = 8
BL = B // NC          # 2 batch rows per core
VS = V // NC          # 4000 vocab rows per core
F32 = mybir.dt.float32
F32R = mybir.dt.float32  # fp32r needs rounded producers; plain fp32 for now
AF = mybir.ActivationFunctionType
MEMC = 256            # B*M=240 padded to 256 (fp32r needs free>=256 for 1cyc/row)
ECH = [(0, 128), (128, 256), (256, 300)]             # E row chunks
EACH = [(0, 128), (128, 256), (256, 301)]            # E+1 (bias row) chunks
HCH = [(0, 128), (128, 256), (256, 384), (384, 512)]

_cache = {}


def _chunked_load(nc, pool, dram, chunks, ncols, tag, dtype=F32R, cols=None):
    # dram is padded to len(chunks)*128 rows; single DMA, chunk-major layout
    nch = len(chunks)
    t_ = pool.tile([128, nch, ncols], dtype, tag=tag)
    c0, c1 = (0, ncols) if cols is None else cols
    src = dram[0 : 128 * nch, c0:c1].rearrange("(c p) n -> p c n", p=128)
    if dtype == F32R:
        src = src.bitcast(F32R)
    nc.sync.dma_start(t_, src)
    return t_


def build_k1(steps=L, tsteps=T, fused=False):
    nc = bass.Bass(trn_type="TRN2", name="cteg_fused" if fused else "cteg_rec",
                   num_devices=NC if fused else None)
    d = {}

    def inp(name, shape):
        d[name] = nc.dram_tensor(name, list(shape), F32, kind="ExternalInput")
        return d[name]

    TB = 2 * tsteps
    # packed per-core activations: cols [0:TB]=topicT, [TB:TB+2L]=essayT,
    # [TB+2L:TB+2L+MEMC]=memT0
    inp("actT", (384, TB + 2 * steps + MEMC))
    inp("enc_xT_f", (384, 4 * H))
    inp("enc_xT_b", (384, 4 * H))
    inp("enc_hT_f", (H, 4 * H))
    inp("enc_hT_b", (H, 4 * H))
    inp("decXT", (384, 4 * H))
    inp("decHT", (H, 4 * H))
    inp("decMT", (384, 4 * H))
    inp("decAT", (H, 4 * H))
    inp("wp1T_a", (640, E))
    inp("wp2T_a", (640, A))
    inp("wepT_a", (640, A))
    inp("wi1T_a", (384, E))
    inp("wmpT", (384, E))
    inp("attn_vT", (A, 1))
    inp("mask_attn", (TB, BL))      # [(t,b), b'] = (b==b')
    inp("mask_memT", (BL, MEMC))    # [b', c] = (c//120==b'), pad cols 0
    if fused:
        d["woT"] = nc.dram_tensor("woT", [640, VS], mybir.dt.float16,
                                  kind="ExternalInput")
        # int8 logits + per-(row, col-group) absmax scales; host dequantizes
        lg = nc.dram_tensor("lg", [B * L, VS], mybir.dt.int8,
                            kind="ExternalOutput")
        lsc = nc.dram_tensor("lsc", [B * L, 2], F32, kind="ExternalOutput")
    else:
        hs = nc.dram_tensor("hs", [steps, BL, H], F32, kind="ExternalOutput")

    with tile.TileContext(nc) as tc:
        with ExitStack() as ctx:
            wp = ctx.enter_context(tc.tile_pool(name="wts", bufs=1))
            sp = ctx.enter_context(tc.tile_pool(name="big", bufs=1))
            stp = ctx.enter_context(tc.tile_pool(name="state", bufs=3))
            rp = ctx.enter_context(tc.tile_pool(name="roll", bufs=4))
            sgp = ctx.enter_context(tc.tile_pool(name="sigp", bufs=2))
            # recurrence-phase pools (PSUM + decoder weights); closed before
            # the fused vocab-projection phase to free PSUM banks and SBUF
            rctx = ExitStack()
            pg = rctx.enter_context(tc.tile_pool(name="psg", bufs=1, space="PSUM"))
            pb = rctx.enter_context(tc.tile_pool(name="psb", bufs=1, space="PSUM"))
            pt = rctx.enter_context(tc.tile_pool(name="pst", bufs=1, space="PSUM"))

            # ---- small resident constants ----
            topicT = _chunked_load(nc, wp, d["actT"], EACH, TB, "topicT",
                                   cols=(0, TB))
            essayT = _chunked_load(nc, wp, d["actT"], EACH, 2 * steps, "essayT",
                                   cols=(TB, TB + 2 * steps))
            HACH = [(0, 128), (128, 256), (256, 384), (384, 512), (512, 513)]
            wp1T = _chunked_load(nc, wp, d["wp1T_a"], HACH, E, "wp1T")
            wp2T = _chunked_load(nc, wp, d["wp2T_a"], HACH, A, "wp2T")
            wepT = _chunked_load(nc, wp, d["wepT_a"], HACH, A, "wepT")
            wi1T = _chunked_load(nc, wp, d["wi1T_a"], EACH, E, "wi1T")
            wmpT = _chunked_load(nc, wp, d["wmpT"], ECH, E, "wmpT")
            attn_vT = wp.tile([A, 1], F32R, tag="attn_vT")
            nc.sync.dma_start(attn_vT, d["attn_vT"][:, :].bitcast(F32R))
            mask_attn = wp.tile([TB, BL], F32, tag="mask_attn")
            nc.sync.dma_start(mask_attn, d["mask_attn"][:, :])
            mask_memT = wp.tile([BL, MEMC], F32, tag="mask_memT")
            nc.sync.dma_start(mask_memT, d["mask_memT"][:, :])
            mask_memTr = mask_memT.bitcast(F32R)

            ident = wp.tile([128, 128], F32, tag="ident")
            make_identity(nc, ident)
            identr = ident.bitcast(F32R)
            ones2f = wp.tile([2, 128], F32, tag="ones2")
            nc.vector.memset(ones2f, 1.0)
            ones2 = ones2f.bitcast(F32R)

            memT = sp.tile([128, 3, MEMC], F32, tag="memT")
            nc.sync.dma_start(
                memT, d["actT"][0:384,
                                TB + 2 * steps : TB + 2 * steps + MEMC].rearrange(
                    "(c p) n -> p c n", p=128))
            memTr = memT.bitcast(F32R)

            h_bm = stp.tile([2, H], F32, tag="h_bm")
            c_bm = stp.tile([2, H], F32, tag="c_bm")
            # enc_outs stored transposed: eoT[:, k, 2t+b] = enc_outs[b, t, 128k+p]
            eoT = sp.tile([128, 4, TB], F32, tag="eoT")
            if fused:
                # hsT_acc[p, k, b*steps+t] = dec h_t[b, 128k+p] (f16 for the
                # fp16 vocab projection; recurrence itself stays fp32)
                hsT_acc = sp.tile([128, 4, BL * steps], mybir.dt.float16,
                                  tag="hsT_acc")

            def lstm_pointwise(gate_ps, cprev, cnext, hnext):
                # gate_ps [2, 4H] flat: i|f|g|o
                sig = sgp.tile([2, 4 * H], F32, tag="sig")
                nc.scalar.activation(sig[:, 0 : 2 * H], gate_ps[:, 0 : 2 * H],
                                     AF.Sigmoid)
                nc.scalar.activation(sig[:, 2 * H : 3 * H],
                                     gate_ps[:, 2 * H : 3 * H], AF.Tanh)
                nc.scalar.activation(sig[:, 3 * H : 4 * H],
                                     gate_ps[:, 3 * H : 4 * H], AF.Sigmoid)
                tmp = rp.tile([2, H], F32, tag="ctmp")
                nc.vector.tensor_mul(cnext, sig[:, H : 2 * H], cprev)
                nc.vector.tensor_mul(tmp, sig[:, 0:H], sig[:, 2 * H : 3 * H])
                nc.vector.tensor_add(cnext, cnext, tmp)
                tc2 = rp.tile([2, H], F32, tag="tc2")
                nc.scalar.activation(tc2, cnext, AF.Tanh)
                nc.vector.tensor_mul(hnext, sig[:, 3 * H : 4 * H], tc2)

            # ================= ENCODER =================
            hfin = {}
            cfin = {}
            with ExitStack() as ectx:
                eps2 = ectx.enter_context(tc.tile_pool(name="encs", bufs=4))
                for dr in ("f", "b"):
                    with ExitStack() as dctx:
                        epd = dctx.enter_context(
                            tc.tile_pool(name=f"encw{dr}", bufs=1))
                        xsb = epd.tile([TB, 4 * H], F32R, tag="xsb")
                        with ExitStack() as xctx:
                            xp = xctx.enter_context(
                                tc.tile_pool(name=f"encx{dr}", bufs=1))
                            xpp = xctx.enter_context(
                                tc.tile_pool(name=f"encxp{dr}", bufs=1,
                                             space="PSUM"))
                            ew = _chunked_load(nc, xp, d[f"enc_xT_{dr}"], EACH,
                                               4 * H, "ew")
                            for hf_ in range(2):
                                xps = xpp.tile([TB, 2 * H], F32, tag="xps")
                                for ki, (r0, r1) in enumerate(EACH):
                                    for c2 in range(2):
                                        cc = 2 * hf_ + c2
                                        nc.tensor.matmul(
                                            xps[:, 512 * c2 : 512 * c2 + 512],
                                            topicT[: r1 - r0, ki, :],
                                            ew[: r1 - r0, ki,
                                               512 * cc : 512 * cc + 512],
                                            start=(ki == 0), stop=(ki == 2))
                                nc.scalar.copy(
                                    xsb.bitcast(F32)[:, 1024 * hf_ :
                                                     1024 * hf_ + 1024], xps)
                        ehw = _chunked_load(
                            nc, epd, d[f"enc_hT_{dr}"],
                            [(128 * k, 128 * k + 128) for k in range(4)],
                            4 * H, "ehw")
                        hT0 = eps2.tile([128, 4, 2], F32R, tag="ehT")
                        nc.vector.memset(hT0.bitcast(F32), 0.0)
                        hT = None
                        cd = eps2.tile([2, H], F32, tag="ecd")
                        nc.vector.memset(cd, 0.0)
                        for s in range(tsteps):
                            t = s if dr == "f" else tsteps - 1 - s
                            tc.strict_bb_all_engine_barrier()
                            gps = pg.tile([2, 4 * H], F32, tag="gps")
                            if s == 0:
                                hT_prev = hT0
                            elif dr == "f":
                                hT_prev = eoT.bitcast(F32R)[
                                    :, :, 2 * (t - 1) : 2 * (t - 1) + 2]
                            else:
                                hT_prev = hT
                            for cc in range(4):
                                cs = slice(512 * cc, 512 * cc + 512)
                                for ki in range(4):
                                    nc.tensor.matmul(
                                        gps[:, cs],
                                        hT_prev[:, ki, :], ehw[:, ki, cs],
                                        start=(ki == 0), stop=False)
                                nc.tensor.matmul(
                                    gps[:, cs],
                                    identr[:TB, 2 * t : 2 * t + 2],
                                    xsb[:, cs],
                                    start=False, stop=True)
                            cnew = eps2.tile([2, H], F32, tag="ecn")
                            hnew = eps2.tile([2, H], F32, tag="ehn")
                            lstm_pointwise(gps, cd, cnew, hnew)
                            cd = cnew
                            tp = pt.tile([128, 8], F32, tag="tp")
                            for k, (r0, r1) in enumerate(HCH):
                                nc.tensor.transpose(
                                    tp[:, 2 * k : 2 * k + 2],
                                    hnew[:, r0:r1], ident[:2, :2])
                            tdst = eoT[:, :, 2 * t : 2 * t + 2]
                            tsrc = tp.rearrange("p (k b) -> p k b", b=2)
                            if dr == "f":
                                nc.vector.tensor_copy(tdst, tsrc)
                            else:
                                nc.vector.tensor_add(tdst, tdst, tsrc)
                            if s < tsteps - 1:
                                if dr == "f":
                                    hT = None  # fwd reads eoT directly
                                else:
                                    hT = eps2.tile([128, 4, 2], F32R, tag="ehT")
                                    nc.vector.tensor_copy(hT.bitcast(F32), tsrc)
                            else:
                                hfin[dr] = hnew
                        cfin[dr] = cd
                nc.vector.tensor_add(h_bm, hfin["f"], hfin["b"])
                nc.vector.tensor_add(c_bm, cfin["f"], cfin["b"])

            # dec weights in a pool opened after encoder pools closed
            H4CH = [(128 * k, 128 * k + 128) for k in range(4)]
            dwp = rctx.enter_context(tc.tile_pool(name="decw", bufs=1))
            decXT = _chunked_load(nc, dwp, d["decXT"], EACH, 4 * H, "decXT")
            decHT = _chunked_load(nc, dwp, d["decHT"], H4CH, 4 * H, "decHT")
            decMT = _chunked_load(nc, dwp, d["decMT"], ECH, 4 * H, "decMT")

            # hcT: chunks 0-3 = hT, 4-7 = cT
            hcT = stp.tile([128, 8, 2], F32R, tag="hcT")
            tp0 = pt.tile([128, 16], F32, tag="tp")
            for k, (r0, r1) in enumerate(HCH):
                nc.tensor.transpose(tp0[:, 2 * k : 2 * k + 2], h_bm[:, r0:r1],
                                    ident[:2, :2])
                nc.tensor.transpose(tp0[:, 8 + 2 * k : 8 + 2 * k + 2],
                                    c_bm[:, r0:r1], ident[:2, :2])
            nc.vector.tensor_copy(hcT.bitcast(F32),
                                  tp0.rearrange("p (k b) -> p k b", b=2))

            tc.strict_bb_all_engine_barrier()
            # ---- precompute phase ----
            TBL = 2 * steps
            P_sb = sp.tile([TB, 4 * H], F32R, tag="P_sb")
            epT_sb = sp.tile([A, TB], F32, tag="epT_sb")
            UT_sb = sp.tile([128, 3, TBL], F32, tag="UT_sb")
            XD_sb = sp.tile([TBL, 4 * H], F32R, tag="XD_sb")
            with ExitStack() as pctx:
                ppre = pctx.enter_context(
                    tc.tile_pool(name="pre", bufs=1, space="PSUM"))
                dap = pctx.enter_context(tc.tile_pool(name="decA", bufs=1))
                for hf_ in range(2):
                    decAT = dap.tile([128, 4, 1024], F32R, tag="decAT")
                    nc.sync.dma_start(
                        decAT,
                        d["decAT"][:, 1024 * hf_ : 1024 * hf_ + 1024].rearrange(
                            "(c p) n -> p c n", p=128).bitcast(F32R))
                    pps = ppre.tile([TB, 2 * H], F32, tag="pre")
                    for ki in range(4):
                        for c2 in range(2):
                            nc.tensor.matmul(
                                pps[:, 512 * c2 : 512 * c2 + 512], eoT.bitcast(F32R)[:, ki, :],
                                decAT[:, ki, 512 * c2 : 512 * c2 + 512],
                                start=(ki == 0), stop=(ki == 3))
                    nc.scalar.copy(
                        P_sb.bitcast(F32)[:, 1024 * hf_ : 1024 * hf_ + 1024], pps)

                # enc_procT [A, TB] (A-major): lhsT = wepT chunks, rhs = eoT (+ones)
                eph = ppre.tile([A, TB], F32, tag="pre")
                for ki in range(4):
                    nc.tensor.matmul(eph, wepT[:, ki, :], eoT.bitcast(F32R)[:, ki, :],
                                     start=(ki == 0), stop=False)
                nc.tensor.matmul(eph, wepT[0:1, 4, :], ones2[0:1, 0:TB],
                                 start=False, stop=True)
                nc.vector.tensor_copy(epT_sb, eph)

                for j, (c0, c1) in enumerate(ECH):
                    ups = ppre.tile([128, TBL], F32, tag="pre")
                    for ki, (r0, r1) in enumerate(EACH):
                        nc.tensor.matmul(ups[: c1 - c0, :],
                                         wi1T[: r1 - r0, ki, c0:c1],
                                         essayT[: r1 - r0, ki, :],
                                         start=(ki == 0), stop=(ki == 2))
                    nc.scalar.copy(UT_sb[: c1 - c0, j, :], ups[: c1 - c0, :])

                for cc in range(4):
                    xps2 = ppre.tile([TBL, H], F32, tag="pre")
                    for ki, (r0, r1) in enumerate(EACH):
                        nc.tensor.matmul(xps2, essayT[: r1 - r0, ki, :],
                                         decXT[: r1 - r0, ki,
                                               512 * cc : 512 * cc + 512],
                                         start=(ki == 0), stop=(ki == 2))
                    nc.scalar.copy(XD_sb.bitcast(F32)[:, 512 * cc : 512 * cc + 512],
                                   xps2)

            pc = rctx.enter_context(tc.tile_pool(name="psc", bufs=1, space="PSUM"))

            # ================= DECODER =================
            for t in range(steps):
                tc.strict_bb_all_engine_barrier()
                # ---- mem write pipeline (h-independent) ----
                candp = pc.tile([128, 3, MEMC], F32, tag="candp")
                for j, (c0, c1) in enumerate(ECH):
                    for ki, (r0, r1) in enumerate(ECH):
                        nc.tensor.matmul(candp[: c1 - c0, j, :],
                                         wmpT[: r1 - r0, ki, c0:c1],
                                         memTr[: r1 - r0, ki, :],
                                         start=(ki == 0), stop=(ki == 2))
                gps_m = pb.tile([2, MEMC], F32, tag="sm")
                for ki, (r0, r1) in enumerate(ECH):
                    nc.tensor.matmul(gps_m, essayT[: r1 - r0, ki, 2 * t : 2 * t + 2],
                                     memTr[: r1 - r0, ki, :],
                                     start=(ki == 0), stop=(ki == 2))
                g_sb = rp.tile([2, MEMC], F32, tag="g_sb")
                nc.scalar.activation(g_sb, gps_m, AF.Sigmoid)
                nc.vector.tensor_mul(g_sb, g_sb, mask_memT)

                tc.strict_bb_all_engine_barrier()
                # ---- mem read: v, sim, mt ----
                vps = pb.tile([2, E], F32, tag="sm")
                for ki in range(4):
                    nc.tensor.matmul(vps, hcT[:, ki, :], wp1T[:, ki, :],
                                     start=(ki == 0), stop=False)
                nc.tensor.matmul(vps, ones2[0:1, 0:2], wp1T[0:1, 4, :],
                                 start=False, stop=True)
                v_bm = rp.tile([2, E], F32, tag="v_bm")
                nc.scalar.activation(v_bm, vps, AF.Tanh)
                vT = rp.tile([128, 3, 2], F32R, tag="vT")
                tpv = pt.tile([128, 6], F32, tag="tp")
                for j, (r0, r1) in enumerate(ECH):
                    nc.tensor.transpose(tpv[: r1 - r0, 2 * j : 2 * j + 2],
                                        v_bm[:, r0:r1], ident[:2, :2])
                for j, (r0, r1) in enumerate(ECH):
                    nc.vector.tensor_copy(vT.bitcast(F32)[: r1 - r0, j, :],
                                          tpv[: r1 - r0, 2 * j : 2 * j + 2])
                sps = pb.tile([2, MEMC], F32, tag="sm")
                for ki, (r0, r1) in enumerate(ECH):
                    nc.tensor.matmul(sps, vT[: r1 - r0, ki, :],
                                     memTr[: r1 - r0, ki, :],
                                     start=(ki == 0), stop=(ki == 2))
                es = rp.tile([2, MEMC], F32, tag="es")
                nc.scalar.activation(es, sps, AF.Exp)
                den = rp.tile([2, 1], F32, tag="den")
                nc.vector.tensor_mul(es, es, mask_memT)
                nc.vector.tensor_reduce(op=mybir.AluOpType.add, out=den,
                                        in_=es, axis=mybir.AxisListType.X)
                nc.vector.reciprocal(den, den)
                nc.vector.tensor_scalar_mul(es, es, den)
                esr = es.bitcast(F32R)

                tc.strict_bb_all_engine_barrier()
                mtT = rp.tile([128, 3, 2], F32R, tag="mtT")
                junk = rp.tile([128, 120], F32, tag="junk")
                for j, (r0, r1) in enumerate(ECH):
                    arep = pb.tile([128, MEMC], F32, tag="sm")
                    nc.tensor.matmul(arep[: r1 - r0, :], ones2[:, : r1 - r0], esr,
                                     start=True, stop=True)
                    for b in range(2):
                        nc.vector.tensor_mul(
                            junk[: r1 - r0, :],
                            memT[: r1 - r0, j, 120 * b : 120 * b + 120],
                            arep[: r1 - r0, 120 * b : 120 * b + 120])
                        nc.vector.tensor_reduce(
                            op=mybir.AluOpType.add,
                            out=mtT.bitcast(F32)[: r1 - r0, j, b : b + 1],
                            in_=junk[: r1 - r0, :], axis=mybir.AxisListType.X)

                tc.strict_bb_all_engine_barrier()
                # ---- attention ----
                qps = pb.tile([A, 2], F32, tag="sm")
                for ki in range(4):
                    nc.tensor.matmul(qps, wp2T[:, ki, :], hcT[:, 4 + ki, :],
                                     start=(ki == 0), stop=False)
                nc.tensor.matmul(qps, wp2T[0:1, 4, :], ones2[0:1, 0:2],
                                 start=False, stop=True)
                qsb = rp.tile([A, 2], F32, tag="qsb")
                nc.vector.tensor_copy(qsb, qps)
                tha = rp.tile([A, TB], F32, tag="tha")
                for b in range(2):
                    nc.scalar.activation(
                        tha.rearrange("a (t b) -> a t b", b=2)[:, :, b],
                        epT_sb.rearrange("a (t b) -> a t b", b=2)[:, :, b],
                        AF.Tanh, bias=qsb[:, b : b + 1], scale=1.0)
                scps = pb.tile([1, TB], F32, tag="sm")
                nc.tensor.matmul(scps, attn_vT, tha.bitcast(F32R),
                                 start=True, stop=True)
                esc = rp.tile([1, TB], F32, tag="esc")
                nc.scalar.activation(esc, scps, AF.Exp)
                escT = pt.tile([TB, 1], F32, tag="tp")
                nc.tensor.transpose(escT, esc, ident[0:1, 0:1])
                escTs = rp.tile([TB, 1], F32, tag="escTs")
                nc.vector.tensor_copy(escTs, escT)
                sms = pb.tile([2, 1], F32, tag="sm")
                nc.tensor.matmul(sms, mask_attn.bitcast(F32R),
                                 escTs.bitcast(F32R), start=True, stop=True)
                rden = rp.tile([2, 1], F32, tag="rden")
                nc.vector.reciprocal(rden, sms)
                rrT = pt.tile([1, 2], F32, tag="tp")
                nc.tensor.transpose(rrT, rden, ident[:2, :2])
                rr_sb = rp.tile([1, 2], F32, tag="rr_sb")
                nc.vector.tensor_copy(rr_sb, rrT)
                rrep = pb.tile([TB, 2], F32, tag="sm")
                nc.tensor.matmul(rrep, ones2[0:1, 0:TB], rr_sb.bitcast(F32R),
                                 start=True, stop=True)
                alBD = rp.tile([TB, BL], F32, tag="alBD")
                nc.vector.tensor_scalar_mul(alBD, mask_attn, escTs)
                nc.vector.tensor_mul(alBD, alBD, rrep)

                tc.strict_bb_all_engine_barrier()
                # ---- gates ----
                gps = pg.tile([2, 4 * H], F32, tag="gps")
                for cc in range(4):
                    cs = slice(512 * cc, 512 * cc + 512)
                    for ki in range(4):
                        nc.tensor.matmul(gps[:, cs],
                                         hcT[:, ki, :], decHT[:, ki, cs],
                                         start=(ki == 0), stop=False)
                    for ki, (r0, r1) in enumerate(ECH):
                        nc.tensor.matmul(gps[:, cs],
                                         mtT[: r1 - r0, ki, :],
                                         decMT[: r1 - r0, ki, cs],
                                         start=False, stop=False)
                    nc.tensor.matmul(gps[:, cs],
                                     alBD.bitcast(F32R), P_sb[:, cs],
                                     start=False, stop=False)
                    nc.tensor.matmul(gps[:, cs],
                                     identr[:TBL, 2 * t : 2 * t + 2],
                                     XD_sb[:, cs],
                                     start=False, stop=True)

                c_new = stp.tile([2, H], F32, tag="c_bm")
                h_new = stp.tile([2, H], F32, tag="h_bm")
                lstm_pointwise(gps, c_bm, c_new, h_new)
                c_bm, h_bm = c_new, h_new
                if not fused:
                    nc.sync.dma_start(hs[t, :, :], h_new)
                hcT = stp.tile([128, 8, 2], F32R, tag="hcT")
                tph = pt.tile([128, 16], F32, tag="tp")
                for k, (r0, r1) in enumerate(HCH):
                    nc.tensor.transpose(tph[:, 2 * k : 2 * k + 2],
                                        h_new[:, r0:r1], ident[:2, :2])
                    nc.tensor.transpose(tph[:, 8 + 2 * k : 8 + 2 * k + 2],
                                        c_new[:, r0:r1], ident[:2, :2])
                nc.vector.tensor_copy(hcT.bitcast(F32),
                                      tph.rearrange("p (k b) -> p k b", b=2))
                if fused:
                    nc.scalar.copy(
                        hsT_acc.rearrange("p k (b t) -> p k t b",
                                          t=steps)[:, :, t, :],
                        tph.rearrange("p (k b) -> p k b", b=2)[:, 0:4, :])

                tc.strict_bb_all_engine_barrier()
                # ---- mem blend: mem += gb * (cand - mem) ----
                for j, (r0, r1) in enumerate(ECH):
                    gb = pb.tile([128, MEMC], F32, tag="sm")
                    nc.tensor.matmul(gb[: r1 - r0, :], ones2[:, : r1 - r0],
                                     g_sb.bitcast(F32R), start=True, stop=True)
                    dd = rp.tile([128, MEMC], F32, tag="dd")
                    for b in range(2):
                        bc = slice(120 * b, 120 * b + 120)
                        nc.vector.tensor_scalar_add(
                            dd[: r1 - r0, bc],
                            candp[: r1 - r0, j, bc],
                            UT_sb[: r1 - r0, j, 2 * t + b : 2 * t + b + 1])
                    nc.vector.tensor_sub(dd[: r1 - r0, 0:240],
                                         dd[: r1 - r0, 0:240],
                                         memT[: r1 - r0, j, 0:240])
                    nc.vector.tensor_mul(dd[: r1 - r0, 0:240],
                                         dd[: r1 - r0, 0:240],
                                         gb[: r1 - r0, 0:240])
                    nc.vector.tensor_add(memT[: r1 - r0, j, 0:240],
                                         memT[: r1 - r0, j, 0:240],
                                         dd[: r1 - r0, 0:240])

            rctx.close()
            if fused:
                F16 = mybir.dt.float16
                LTOK = BL * steps          # local token cols (128)
                dpool = ctx.enter_context(
                    tc.tile_pool(name="dramp", bufs=1, space="DRAM"))
                hs_locT = dpool.tile([H, LTOK], F16, tag="hs_locT")
                hs_allT = dpool.tile([NC * H, LTOK], F16, tag="hs_allT")
                nc.gpsimd.dma_start(
                    hs_locT.rearrange("(k p) n -> p k n", p=128), hsT_acc)
                nc.gpsimd.collective_compute(
                    "AllGather", mybir.AluOpType.bypass,
                    replica_groups=[list(range(NC))],
                    ins=[hs_locT.opt()], outs=[hs_allT.opt()])

                vw = ctx.enter_context(tc.tile_pool(name="vw", bufs=1))
                vo = ctx.enter_context(tc.tile_pool(name="vo", bufs=3))
                vp = ctx.enter_context(
                    tc.tile_pool(name="vp", bufs=2, space="PSUM"))
                wT = vw.tile([128, 5, VS], F16, tag="wT")
                nc.sync.dma_start(
                    wT, d["woT"][0:640, :].rearrange("(c p) n -> p c n", p=128))
                hT = vw.tile([128, 5, B * L], F16, tag="hT")
                nc.vector.memset(hT[0:1, 4, :], 1.0)
                for c in range(NC):
                    nc.gpsimd.dma_start(
                        hT[:, 0:4, LTOK * c : LTOK * (c + 1)],
                        hs_allT[H * c : H * (c + 1), :].rearrange(
                            "(k p) n -> p k n", p=128))
                chunks = [(o, min(512, VS - o)) for o in range(0, VS, 512)]
                groups = [chunks[i : i + 4] for i in range(0, len(chunks), 4)]
                NBG = 2048
                for mb in range(B * L // 128):
                    for gi, grp in enumerate(groups):
                        g0 = grp[0][0]
                        gw = grp[-1][0] + grp[-1][1] - g0
                        ps = vp.tile([128, NBG], F32, tag="ps")
                        for k in range(5):
                            kw = 128 if k < 4 else 1
                            for (o, w_) in grp:
                                nc.tensor.matmul(
                                    ps[:, o - g0 : o - g0 + w_],
                                    hT[:kw, k, 128 * mb : 128 * mb + 128],
                                    wT[:kw, k, o : o + w_],
                                    start=(k == 0), stop=(k == 4))
                        amax = vo.tile([128, 1], F32, tag="amax")
                        nc.vector.tensor_reduce(
                            op=mybir.AluOpType.max, out=amax,
                            in_=ps[:, :gw], axis=mybir.AxisListType.X,
                            apply_absolute_value=True)
                        inv = vo.tile([128, 1], F32, tag="inv")
                        nc.vector.reciprocal(inv, amax)
                        fac = vo.tile([128, 1], F32, tag="fac")
                        nc.scalar.activation(fac, inv, AF.Copy, scale=127.0)
                        ot = vo.tile([128, NBG], mybir.dt.int8, tag="ot")
                        nc.scalar.activation(ot[:, :gw], ps[:, :gw], AF.Copy,
                                             scale=fac)
                        nc.sync.dma_start(
                            lg[128 * mb : 128 * mb + 128, g0 : g0 + gw],
                            ot[:, :gw])
                        nc.sync.dma_start(
                            lsc[128 * mb : 128 * mb + 128, gi : gi + 1], amax)
    return nc


def build_k2():
    F16 = mybir.dt.float16
    nc = bass.Bass(trn_type="TRN2", name="cteg_logits")
    hsT = nc.dram_tensor("hsT", [640, B * L], F32, kind="ExternalInput")
    woT = nc.dram_tensor("woT", [640, VS], F32, kind="ExternalInput")
    out = nc.dram_tensor("lg", [B * L, VS], F16, kind="ExternalOutput")
    NBG = 2048
    with tile.TileContext(nc) as tc:
        with ExitStack() as ctx:
            wpo = ctx.enter_context(tc.tile_pool(name="w", bufs=1))
            op = ctx.enter_context(tc.tile_pool(name="o", bufs=3))
            pp = ctx.enter_context(tc.tile_pool(name="p", bufs=2, space="PSUM"))
            hT = wpo.tile([128, 5, B * L], F32R, tag="hT")
            nc.sync.dma_start(
                hT, hsT[0:640, :].rearrange("(c p) n -> p c n", p=128).bitcast(F32R))
            wT = wpo.tile([128, 5, VS], F32R, tag="wT")
            nc.sync.dma_start(
                wT, woT[0:640, :].rearrange("(c p) n -> p c n", p=128).bitcast(F32R))
            chunks = [(o, min(512, VS - o)) for o in range(0, VS, 512)]
            groups = [chunks[i : i + 4] for i in range(0, len(chunks), 4)]
            for mb in range(B * L // 128):
                for grp in groups:
                    g0 = grp[0][0]
                    gw = grp[-1][0] + grp[-1][1] - g0
                    ps = pp.tile([128, NBG], F32, tag="ps")
                    for k in range(5):
                        kw = 128 if k < 4 else 1
                        for (o, w_) in grp:
                            nc.tensor.matmul(
                                ps[:, o - g0 : o - g0 + w_],
                                hT[:kw, k, 128 * mb : 128 * mb + 128],
                                wT[:kw, k, o : o + w_],
                                start=(k == 0), stop=(k == 4))
                    ot = op.tile([128, NBG], F16, tag="ot")
                    nc.scalar.copy(ot[:, :gw], ps[:, :gw])
                    nc.sync.dma_start(
                        out[128 * mb : 128 * mb + 128, g0 : g0 + gw],
                        ot[:, :gw])
    return nc


K1_WEIGHT_KEYS = (
    "enc_Wih_f", "enc_b_f", "enc_Wih_b", "enc_b_b", "enc_Whh_f", "enc_Whh_b",
    "dec_Wih", "dec_b", "dec_Whh", "Wp1", "bp1", "Wp2", "bp2", "Wep", "bep",
    "Wi1", "bi1", "Wmp", "bmp", "attn_v")


def _prep_shared(inputs):
    """Replicated k1 weight tensors (host layout/padding)."""
    f = lambda x: np.ascontiguousarray(np.asarray(x), dtype=np.float32)
    wih = f(inputs["dec_Wih"])
    shared = {
        "enc_xT_f": np.vstack([f(inputs["enc_Wih_f"]).T, f(inputs["enc_b_f"])[None]]),
        "enc_xT_b": np.vstack([f(inputs["enc_Wih_b"]).T, f(inputs["enc_b_b"])[None]]),
        "enc_hT_f": f(inputs["enc_Whh_f"]).T.copy(),
        "enc_hT_b": f(inputs["enc_Whh_b"]).T.copy(),
        "decXT": np.vstack([wih[:, :E].T, f(inputs["dec_b"])[None]]),
        "decAT": wih[:, E : E + H].T.copy(),
        "decMT": wih[:, E + H :].T.copy(),
        "decHT": f(inputs["dec_Whh"]).T.copy(),
        "wp1T_a": np.vstack([f(inputs["Wp1"]).T, f(inputs["bp1"])[None]]),
        "wp2T_a": np.vstack([f(inputs["Wp2"]).T, f(inputs["bp2"])[None]]),
        "wepT_a": np.vstack([f(inputs["Wep"]).T, f(inputs["bep"])[None]]),
        "wi1T_a": np.vstack([f(inputs["Wi1"]).T,
                             (f(inputs["bi1"]) + f(inputs["bmp"]))[None]]),
        "wmpT": f(inputs["Wmp"]).T.copy(),
        "attn_vT": f(inputs["attn_v"])[:, None].copy(),
    }
    mask_attn = np.zeros((2 * T, BL), np.float32)
    for t in range(T):
        for b in range(BL):
            mask_attn[2 * t + b, b] = 1.0
    shared["mask_attn"] = mask_attn
    mask_memT = np.zeros((BL, MEMC), np.float32)
    for b in range(BL):
        mask_memT[b, 120 * b : 120 * (b + 1)] = 1.0
    shared["mask_memT"] = mask_memT
    pad_to = {"enc_xT_f": 384, "enc_xT_b": 384, "decXT": 384, "decMT": 384,
              "wp1T_a": 640, "wp2T_a": 640, "wepT_a": 640, "wi1T_a": 384,
              "wmpT": 384}
    for k, rows in pad_to.items():
        v = shared[k]
        shared[k] = np.pad(v, ((0, rows - v.shape[0]), (0, 0)))
    return {k: np.ascontiguousarray(v, np.float32) for k, v in shared.items()}


def _prep_data(inputs):
    """Per-core embedding-gathered activations, packed [NC*384, TB+2L+MEMC]."""
    emb = np.ascontiguousarray(np.asarray(inputs["embedding"]), np.float32)
    topic = np.asarray(inputs["topic"]).astype(np.int64)
    essay = np.asarray(inputs["essay_input"]).astype(np.int64)
    mems = np.asarray(inputs["mems"]).astype(np.int64)
    te = emb[topic]          # [B, T, E]
    ee = emb[essay]          # [B, L, E]
    me = emb[mems]           # [B, M, E]

    TB = 2 * T
    act = np.zeros((NC, 384, TB + 2 * L + MEMC), np.float32)
    act[:, E, 0 : TB + 2 * L] = 1.0
    # [B,S,E] -> per-core [E, 2*S] with (t,b) interleave on cols
    act[:, :E, 0:TB] = np.moveaxis(
        te.reshape(NC, BL, T, E), (1, 2, 3), (3, 2, 1)).reshape(NC, E, 2 * T)
    act[:, :E, TB : TB + 2 * L] = np.moveaxis(
        ee.reshape(NC, BL, L, E), (1, 2, 3), (3, 2, 1)).reshape(NC, E, 2 * L)
    act[:, :E, TB + 2 * L : TB + 2 * L + 2 * M] = np.moveaxis(
        me.reshape(NC, BL, M, E), (1, 2, 3), (2, 3, 1)).reshape(NC, E, 2 * M)
    return {"actT": act.reshape(NC * 384, TB + 2 * L + MEMC)}


def _prep_wout(inputs, dtype=np.float16):
    """Vocab-sharded transposed output projection, concat over cores."""
    wo = np.asarray(inputs["Wout"], np.float32)
    bo = np.asarray(inputs["bout"], np.float32)
    woT = np.zeros((NC, 640, VS), dtype)
    woT[:, :H] = wo.reshape(NC, VS, H).transpose(0, 2, 1)
    woT[:, H] = bo.reshape(NC, VS)
    return np.ascontiguousarray(woT).reshape(NC * 640, VS)


def _split_multi_waits(bir_json):
    """walrus in this env accepts at most ONE sync wait per instruction
    (S3_LW/CTRL_NO etc. reject more). Hoist extra waits onto same-engine
    NoOps inserted immediately before the instruction — sequencers execute
    in order, so the happens-before relation is preserved."""
    import json

    d = json.loads(bir_json)
    cnt = [0]
    for f in d["functions"]:
        for bb in f["blocks"]:
            out = []
            for inst in bb["instructions"]:
                si = inst.get("sync_info") or {}
                waits = si.get("on_wait") or []
                if len(waits) > 1 and inst["opcode"] != "ISA":
                    for w in waits[:-1]:
                        cnt[0] += 1
                        out.append({
                            "debug": inst.get("debug", 0),
                            "engine": inst["engine"],
                            "ins": [],
                            "outs": [],
                            "name": f"{inst['name']}-w{cnt[0]}",
                            "opcode": "NoOp",
                            "sync_info": {"on_update": [], "on_wait": [w]},
                        })
                    si["on_wait"] = [waits[-1]]
                    inst["sync_info"] = si
                out.append(inst)
            bb["instructions"] = out
    return json.dumps(d).encode()


def _patch_compile():
    import concourse.bass_utils as bu
    import concourse.bass2jax as b2j
    if getattr(bu, "_wait_patched", False):
        return
    orig = bu.compile_bir_kernel

    def patched(bir_json, tmpdir, neff_name="file.neff"):
        return orig(_split_multi_waits(bir_json), tmpdir, neff_name)

    bu.compile_bir_kernel = patched
    b2j.compile_bir_kernel = patched
    bu._wait_patched = True


# ---------------- persistent runner ----------------

import zlib


def _crc(*arrs):
    h = 0
    for a in arrs:
        a = np.ascontiguousarray(np.asarray(a))
        h = zlib.crc32(a.view(np.uint8).reshape(-1), h)
    return h


def _mesh():
    import jax
    from jax.sharding import Mesh
    if "mesh" not in _cache:
        devs = jax.devices()[:NC]
        assert len(devs) == NC
        _cache["mesh"] = Mesh(np.asarray(devs), ("core",))
    return _cache["mesh"]


def _meta(nc_obj):
    import jax
    partition_name = (nc_obj.partition_id_tensor.name
                      if nc_obj.partition_id_tensor else None)
    in_names, out_names, out_avals = [], [], []
    for alloc in nc_obj.m.functions[0].allocations:
        if not isinstance(alloc, mybir.MemoryLocationSet):
            continue
        name = alloc.memorylocations[0].name
        if alloc.kind == "ExternalInput":
            if name != partition_name:
                in_names.append(name)
        elif alloc.kind == "ExternalOutput":
            out_names.append(name)
            out_avals.append(jax.core.ShapedArray(
                tuple(alloc.tensor_shape), mybir.dt.np(alloc.dtype)))
    return in_names, out_names, out_avals, partition_name


def _make_fn(nc_obj, core_sharded_names):
    """Jitted SPMD launcher for a finalized bass module. Outputs are fully
    written by our kernels, so no donated zero buffers are passed."""
    import jax
    from jax.experimental.shard_map import shard_map
    from jax.sharding import PartitionSpec as P
    from concourse.bass2jax import _bass_exec_p, partition_id_tensor

    in_names, out_names, out_avals, partition_name = _meta(nc_obj)
    bind_names = tuple(in_names) + ((partition_name,) if partition_name else ())

    def _body(*args):
        operands = list(args)
        if partition_name:
            operands.append(partition_id_tensor())
        outs = _bass_exec_p.bind(
            *operands, out_avals=tuple(out_avals), in_names=bind_names,
            out_names=tuple(out_names), lowering_input_output_aliases=(),
            sim_require_finite=True, sim_require_nnan=True, nc=nc_obj)
        return tuple(outs)

    mesh = _mesh()
    in_specs = tuple(P("core") if n in core_sharded_names else P()
                     for n in in_names)
    out_specs = (P("core"),) * len(out_names)
    fn = jax.jit(shard_map(_body, mesh=mesh, in_specs=in_specs,
                           out_specs=out_specs, check_rep=False))
    return fn, in_names, out_names


def _upload_rep(np_map):
    """Host -> dev0 (1x over the wire) -> all-device replicate (D2D)."""
    import jax
    from jax.sharding import NamedSharding, PartitionSpec as P
    mesh = _mesh()
    vals = list(np_map.values())
    on0 = jax.device_put(vals, jax.devices()[0])
    jax.block_until_ready(on0)
    rep = jax.device_put(on0, NamedSharding(mesh, P()))
    jax.block_until_ready(rep)
    return dict(zip(np_map.keys(), rep))


def _upload_shard(np_list, block=True):
    import jax
    from jax.sharding import NamedSharding, PartitionSpec as P
    s = NamedSharding(_mesh(), P("core"))
    out = jax.device_put(np_list, s)
    if block:
        jax.block_until_ready(out)
    return out


def _get_act_fn():
    """emb [V, E] replicated + index tensors -> actT [NC*384, 400] core-sharded,
    computed on device (saves shipping the gathered embeddings per call)."""
    import jax
    import jax.numpy as jnp
    from functools import partial
    from jax.sharding import NamedSharding, PartitionSpec as P
    if "act_fn" in _cache:
        return _cache["act_fn"]
    s_core = NamedSharding(_mesh(), P("core"))
    TBv = 2 * T
    NCOL = TBv + 2 * L + MEMC

    @partial(jax.jit, out_shardings=s_core)
    def g(emb, topic, essay, mems):
        te = emb[topic].reshape(NC, BL, T, E).transpose(0, 3, 2, 1)
        ee = emb[essay].reshape(NC, BL, L, E).transpose(0, 3, 2, 1)
        me = emb[mems].reshape(NC, BL, M, E).transpose(0, 3, 1, 2)
        datab = jnp.concatenate([
            te.reshape(NC, E, TBv), ee.reshape(NC, E, 2 * L),
            me.reshape(NC, E, 2 * M),
            jnp.zeros((NC, E, MEMC - 2 * M), jnp.float32)], axis=2)
        ones_row = jnp.concatenate([
            jnp.ones((NC, 1, TBv + 2 * L), jnp.float32),
            jnp.zeros((NC, 1, MEMC), jnp.float32)], axis=2)
        act = jnp.concatenate(
            [datab, ones_row, jnp.zeros((NC, 384 - E - 1, NCOL), jnp.float32)],
            axis=1)
        return act.reshape(NC * 384, NCOL)

    _cache["act_fn"] = g
    return g


def _act_on_device(inputs):
    import jax
    embkey = _crc(inputs["embedding"])
    if _cache.get("embkey") != embkey:
        emb = np.ascontiguousarray(np.asarray(inputs["embedding"]), np.float32)
        _cache["emb_dev"] = _upload_rep({"emb": emb})["emb"]
        _cache["embkey"] = embkey
    idx = [np.ascontiguousarray(np.asarray(inputs[k]).astype(np.int32))
           for k in ("topic", "essay_input", "mems")]
    from jax.sharding import NamedSharding, PartitionSpec as P
    idx_dev = jax.device_put(idx, NamedSharding(_mesh(), P()))
    return _get_act_fn()(_cache["emb_dev"], *idx_dev)


def _get_gather_fn():
    """hs [NC*L, BL, H] sharded-by-core -> hsT [640, B*L] replicated."""
    import jax
    import jax.numpy as jnp
    from functools import partial
    from jax.sharding import NamedSharding, PartitionSpec as P
    if "gather_fn" in _cache:
        return _cache["gather_fn"]
    s_rep = NamedSharding(_mesh(), P())

    @partial(jax.jit, out_shardings=s_rep)
    def g(hs):
        x = hs.reshape(NC, L, BL, H).transpose(0, 2, 1, 3).reshape(B * L, H)
        hT = x.T
        ones = jnp.ones((1, B * L), jnp.float32)
        pad = jnp.zeros((640 - H - 1, B * L), jnp.float32)
        return jnp.concatenate([hT, ones, pad], axis=0)

    _cache["gather_fn"] = g
    return g


def _upload_weights(inputs, wo_dtype=np.float16):
    k1key = _crc(*(inputs[k] for k in K1_WEIGHT_KEYS))
    if _cache.get("k1key") != k1key:
        _cache["k1w"] = _upload_rep(_prep_shared(inputs))
        _cache["k1key"] = k1key
    k2key = (_crc(inputs["Wout"], inputs["bout"]), np.dtype(wo_dtype).str)
    if _cache.get("k2key") != k2key:
        _cache["woT"] = _upload_shard([_prep_wout(inputs, wo_dtype)])[0]
        _cache["k2key"] = k2key


def _assemble(lg):
    lg_np = np.asarray(lg).reshape(NC, B * L, VS)       # float16
    out = np.empty((B * L, V), np.float32)
    for c in range(NC):
        out[:, VS * c : VS * (c + 1)] = lg_np[c]
    return out.reshape(B, L, V)


def _kernel_fused(inputs):
    from concurrent.futures import ThreadPoolExecutor
    if "kf_fn" not in _cache:
        _cache["kf"] = build_k1(fused=True)
        _cache["kf_fn"] = _make_fn(_cache["kf"], {"actT", "woT"})
    if "pool" not in _cache:
        _cache["pool"] = ThreadPoolExecutor(8)
    if _cache.get("act_broken"):
        act_dev = None
    else:
        try:
            act_dev = _act_on_device(inputs)    # async; overlaps weight crc
        except Exception:
            import traceback
            traceback.print_exc()
            _cache["act_broken"] = True
            act_dev = None
    if act_dev is None:
        data = _prep_data(inputs)
        (act_dev,) = _upload_shard([data["actT"]], block=False)
    _upload_weights(inputs, np.float16)
    fn, innames, _ = _cache["kf_fn"]
    ops = [act_dev if n == "actT"
           else (_cache["woT"] if n == "woT" else _cache["k1w"][n])
           for n in innames]
    lg, lsc = fn(*ops)
    sc_fut = _cache["pool"].submit(np.asarray, lsc)
    lg_np = np.asarray(lg).reshape(NC, B * L, VS)       # int8
    sc_np = np.asarray(sc_fut.result()).reshape(NC, B * L, 2)
    sc_np = sc_np.astype(np.float32) / 127.0
    out = np.empty((B * L, V), np.float32)

    def _deq(c):
        for gi, (g0, g1) in enumerate(((0, 2048), (2048, VS))):
            np.multiply(lg_np[c][:, g0:g1], sc_np[c][:, gi : gi + 1],
                        out=out[:, VS * c + g0 : VS * c + g1])
    list(_cache["pool"].map(_deq, range(NC)))
    return out.reshape(B, L, V)


def _kernel_split(inputs):
    if "k1_fn" not in _cache:
        _cache["k1"] = build_k1()
        _cache["k1_fn"] = _make_fn(_cache["k1"], {"actT"})
    if "k2_fn" not in _cache:
        _cache["k2"] = build_k2()
        _cache["k2_fn"] = _make_fn(_cache["k2"], {"woT"})
    _upload_weights(inputs, np.float32)
    data = _prep_data(inputs)
    ddev = dict(zip(data.keys(), _upload_shard(list(data.values()))))

    fn1, in1, _ = _cache["k1_fn"]
    ops1 = [ddev[n] if n in ddev else _cache["k1w"][n] for n in in1]
    (hs,) = fn1(*ops1)

    hsT = _get_gather_fn()(hs)

    fn2, in2, _ = _cache["k2_fn"]
    ops2 = [hsT if n == "hsT" else _cache["woT"] for n in in2]
    (lg,) = fn2(*ops2)
    return _assemble(lg)


def kernel(**inputs):
    _patch_compile()
    from concourse.bass2jax import install_neuronx_cc_hook
    install_neuronx_cc_hook()

    if not _cache.get("fused_broken"):
        try:
            return _kernel_fused(inputs)
        except Exception:
            import traceback
            traceback.print_exc()
            _cache["fused_broken"] = True
    return _kernel_split(inputs)

